# revision 60
# baseline (speedup 1.0000x reference)
"""AnchorHeadBase forward+loss as a distributed Bass kernel on 8 TRN2 NeuronCores.

Reference computation:
  cls_pred = conv1x1(inputs, w_cls)  # [B, 24, H, W]
  reg_pred = conv1x1(inputs, w_reg)  # [B, 42, H, W]
  sample anchors at pos_ids/neg_ids, softmax-CE + smooth-L1 -> scalar loss.

Key identity: with pred = concat(cls_pred, reg_pred) viewed as [B, 66, HW],
the sampled value for channel-group g (cls class ci -> g=ci, reg code j ->
g=4+j) of anchor id a in batch b is pred[b, 6*g + a//HW, a % HW].

Sharding: data-parallel over batch + spatial halves: core = 2*b + (pos >= HW/2).
Each core runs the dense GEMM over its [512, 17600] shard (inputs/weights cast
to fp8e4m3 on host; f32 PSUM accumulation; output channels padded to 128 so the
compiler's fast-weight-load path triggers), keeps the [80, 17600] f32 pred map
in SBUF, gathers its sampled columns with gpsimd.ap_gather, selects the
per-sample anchor offset k with host-built 0/1 masks + a tiny selection matmul,
and computes softmax-CE / smooth-L1 partial sums on device. Host sums the 8
per-core partials (the unshard step) into the scalar loss.

Pad samples are neutralized without a validity mask: bias row 79 is 1 and
kmask[79, pad] = ln(1/4), so a pad sample's class logits are all ln(1/4),
logsumexp = 0, picked = 0; its reg rows are all 0 so smooth-L1 is 0.
"""

import os
import sys

sys.path.insert(0, "/opt/trn_rl_repo")

import numpy as np
import ml_dtypes

import concourse.bass as bass
import concourse.mybir as mybir
import concourse.tile as tile
from concourse import bacc
from concourse.bass_utils import run_bass_kernel_spmd


def _ensure_ntff_hook():
    """bass_utils' trace path needs antenv.axon_hooks; some containers lack the
    module (boot degrades silently and no hook gets registered). Install a
    holder module and register the ctypes-based NTFF hook ourselves."""
    import types

    try:
        from antenv import axon_hooks  # noqa: F401
    except ImportError:
        import antenv

        m = types.ModuleType("antenv.axon_hooks")
        m._hook = None

        def set_axon_ntff_profile_hook(h, _m=m):
            _m._hook = h

        def get_axon_ntff_profile_hook(_m=m):
            return _m._hook

        m.set_axon_ntff_profile_hook = set_axon_ntff_profile_hook
        m.get_axon_ntff_profile_hook = get_axon_ntff_profile_hook
        sys.modules["antenv.axon_hooks"] = m
        antenv.axon_hooks = m
    from antenv import axon_hooks
    if axon_hooks.get_axon_ntff_profile_hook() is None:
        try:
            from trn_agent_boot.trn_boot import _ntff_profile_via_ctypes
            so = "/opt/axon/libaxon_pjrt.so"
            if os.path.exists(so):
                axon_hooks.set_axon_ntff_profile_hook(_ntff_profile_via_ctypes(so))
        except Exception:
            pass

# ---- problem constants (hardcoded; must match the reference) ----
B, C_IN = 4, 512
NX, NY = 200, 176
HW = NX * NY              # 35200
HALF = HW // 2            # 17600 columns per core
N_CORES = 8
NUM_CLS, CODE, AGRP = 4, 7, 6
NCH = NUM_CLS * AGRP + CODE * AGRP   # 66 output channels (24 cls + 42 reg)
NCHP = 80                            # gather partition dim (%16 for ap_gather)
NCHM = 128                           # matmul M dim (128 -> fast weight load)
NGRP = NUM_CLS + CODE                # 11 channel groups
M_POS, M_NEG = 512, 4096
N_SAMP = M_NEG + M_POS               # 4608 cls samples (pos samples carry reg)
CLS_W, REG_W = 1.0, 2.0

# DMA slab widths: small first slabs so the PE starts early, then 4096-wide
# (512 KiB fp8 per k-chunk) steady-state slabs.
SLABS = [1024, 2048, 4096, 4096, 4096, 2240]
assert sum(SLABS) == HALF
# gather regions: early regions get their own pred tile + gather right after
# their evictions land (keeps the GPSIMD Q7s warm through the DMA-bound phase);
# every extended GPSIMD instruction pays ~2.4 us dispatch latency, so the tail
# of the map is ONE region with ONE gather instead of several trailing ones
GTILES = [1024, 2048, 2048, 2048, 2048, 4096, 4288]
assert sum(GTILES) == HALF
NTILE = 512                          # matmul moving free dim / PSUM bank

FP8 = mybir.dt.float8e4
FP8_NP = ml_dtypes.float8_e4m3
F32 = mybir.dt.float32
BF16 = mybir.dt.bfloat16

LAST_RESULT = None  # BassKernelResults of the most recent kernel() call


def _ceil_to(x, m):
    return (x + m - 1) // m * m


def _patch_fast_tail():
    """Trim the Tile end-of-kernel epilogue: skip the semaphore-clear pass and
    the second all-engine barrier. Safe for single-execution NEFFs (each
    kernel() call compiles, loads and runs the NEFF exactly once; the runtime
    zeroes semaphores at load)."""
    if getattr(tile.TileContext, "_fast_tail", False):
        return
    orig_dab = tile.TileContext._drain_and_barrier

    def fast(self, tick_clock, wait_clock):
        nc = self.nc
        orig_clear = nc.clear_and_free_semaphores
        orig_barrier = nc.all_engine_barrier
        calls = [0]

        def barrier_once(*a, **k):
            calls[0] += 1
            if calls[0] == 1:
                return orig_barrier(*a, **k)
            return None

        nc.clear_and_free_semaphores = lambda sems: None
        nc.all_engine_barrier = barrier_once
        try:
            orig_dab(self, tick_clock, wait_clock)
        finally:
            nc.clear_and_free_semaphores = orig_clear
            nc.all_engine_barrier = orig_barrier

    tile.TileContext._drain_and_barrier = fast
    tile.TileContext._fast_tail = True


def _patch_ldw_opt():
    """Flip walrus's --enable-ldw-opt on (dedupes back-to-back LDWEIGHTS of
    the same stationary operand)."""
    from concourse import bass_utils as bu
    if getattr(bu, "_ldw_patched", False):
        return
    orig = bu.run_command

    def patched(argv, **kw):
        argv = [a.replace("--enable-ldw-opt=false", "--enable-ldw-opt=true")
                if isinstance(a, str) else a for a in argv]
        return orig(argv, **kw)

    bu.run_command = patched
    bu._ldw_patched = True


def _host_prep(inputs, w_cls, b_cls, w_reg, b_reg, reg_labels, pos_ids, neg_ids,
               cls_labels):
    """Shard inputs, cast to fp8, and build per-core gather/mask tensors."""
    x = np.asarray(inputs, np.float32).reshape(B, C_IN, HW)
    W = np.concatenate([np.asarray(w_cls, np.float32),
                        np.asarray(w_reg, np.float32)], axis=0)     # [66, 512]
    bias = np.concatenate([np.asarray(b_cls, np.float32),
                           np.asarray(b_reg, np.float32)], axis=0)  # [66]
    W_pad = np.zeros((NCHM, C_IN), np.float32)
    W_pad[:NCH] = W
    bias_pad = np.zeros((NCHP, 1), np.float32)
    bias_pad[:NCH, 0] = bias
    bias_pad[79, 0] = 1.0            # pad-sample logsumexp neutralizer

    # lhsT layout: [128, 4, 128] fp8 -- wT[p, k, m] = W_pad[m, 128*k + p]
    wT = np.ascontiguousarray(
        W_pad.T.reshape(4, 128, NCHM).transpose(1, 0, 2)).astype(FP8_NP)

    # selection matrix (bf16): SEL[6*g + k, g] = 1; row 79 feeds the pad
    # neutralizer into every class logit.
    sel = np.zeros((NCHP, NGRP), np.float32)
    for g in range(NGRP):
        for k in range(AGRP):
            sel[6 * g + k, g] = 1.0
    sel[79, 0:NUM_CLS] = 1.0
    sel = sel.astype(ml_dtypes.bfloat16)

    # ---- partition the 4608 samples (neg first, then pos) by owning core ----
    pos_ids = np.asarray(pos_ids)
    neg_ids = np.asarray(neg_ids)
    cls_labels = np.asarray(cls_labels)
    reg_labels = np.asarray(reg_labels, np.float32)

    all_b = np.concatenate([neg_ids[:, 0], pos_ids[:, 0]]).astype(np.int64)
    all_a = np.concatenate([neg_ids[:, 1], pos_ids[:, 1]]).astype(np.int64)
    k_of = all_a // HW                      # anchor offset within pixel, 0..5
    pos_of = all_a % HW                     # spatial position
    core_of = 2 * all_b + (pos_of >= HALF)
    col_of = pos_of % HALF                  # column within the core's shard
    is_pos = np.arange(N_SAMP) >= M_NEG
    label = cls_labels.astype(np.int64)

    # samples are gathered per gather-tile (right after that tile's pred
    # columns land in SBUF), so bucket by (core, gtile)
    gt_off = np.cumsum([0] + GTILES[:-1])
    gt_id_of = np.searchsorted(np.cumsum(GTILES), col_of, side="right")

    NGT = len(GTILES)
    bucket_counts = np.zeros((N_CORES, NGT), np.int64)
    for c in range(N_CORES):
        for si in range(NGT):
            bucket_counts[c, si] = int(
                ((core_of == c) & (gt_id_of == si)).sum())
    # shared padded bucket sizes (same graph on every core); multiples of 32 so
    # every bucket's wrapped-index slice stays 4-byte aligned for the ucode's
    # 32-bit index reads
    bsize = [max(32, _ceil_to(int(bucket_counts[:, si].max()), 32))
             for si in range(NGT)]
    boff = np.cumsum([0] + bsize[:-1])
    NS = int(sum(bsize))
    meta = {"bsize": bsize}

    in_maps = []
    slab_offs = np.cumsum([0] + SLABS[:-1])
    for c in range(N_CORES):
        b_idx, half = c // 2, c % 2
        xs = np.ascontiguousarray(
            x[b_idx, :, half * HALF:(half + 1) * HALF]).astype(FP8_NP)
        # tilt per slab so each slab's [128, 4, w] SBUF tile is one contiguous
        # DRAM region (one dma_start with 16 KiB/partition descriptors)
        xt = np.empty((128, 4 * HALF), FP8_NP)
        for soff, w in zip(slab_offs, SLABS):
            for k in range(4):
                xt[:, 4 * soff + k * w:4 * soff + (k + 1) * w] = \
                    xs[128 * k:128 * (k + 1), soff:soff + w]

        # slot each sample into its gtile bucket (order within bucket is
        # arbitrary)
        jc = np.nonzero(core_of == c)[0]
        order = np.argsort(gt_id_of[jc], kind="stable")
        j = jc[order]
        sl = gt_id_of[j]
        s = np.zeros(len(j), np.int64)      # sample slot within [0, NS)
        for si in range(NGT):
            m = sl == si
            s[m] = boff[si] + np.arange(int(m.sum()))
        n = len(j)
        cols = col_of[j]
        ks = k_of[j]

        # gather indices are gtile-relative; wrapped per 16 partitions
        gidx16 = np.zeros((16, NS // 16), np.int16)
        gidx16[s % 16, s // 16] = (cols - gt_off[sl]).astype(np.int16)
        gidx = np.tile(gidx16, (NCHP // 16, 1))

        kmask = np.zeros((NCHP, NS), np.float32)
        for g in range(NUM_CLS):
            kmask[6 * g + ks, s] = 1.0
        jp = is_pos[j]
        sp = s[jp]
        for g in range(CODE):
            kmask[24 + 6 * g + ks[jp], sp] = 1.0
        pad = np.ones(NS, bool)
        pad[s] = False
        kmask[79, pad] = float(np.log(1.0 / NUM_CLS))  # pad neutralizer

        clab = np.zeros((NUM_CLS, NS), np.float32)
        clab[label[j], s] = 1.0

        reglab = np.zeros((CODE, NS), np.float32)
        reglab[:, sp] = reg_labels[j[jp] - M_NEG].T

        in_maps.append({
            "x": xt,
            "wT": wT,
            "bias": bias_pad,
            "sel": sel,
            "gidx": gidx,
            "kmask": kmask,
            "clab": clab,
            "reglab": reglab,
        })
    return in_maps, NS, meta


def _patch_act_tables():
    """Force the act-table pass to pick the one set covering every function we
    use (ln/exp/relu/abs/square/copy/identity) so exactly one table load is
    emitted, early. Ids are positional, so blank the other sets instead of
    filtering them out."""
    if getattr(bacc, "_act_tables_patched", False):
        return
    orig = bacc.get_activation_tables

    def patched(module_arch):
        tabs = dict(orig(module_arch))
        keep = "natural_log_exp_and_others"
        if keep in tabs:
            tabs = {k: (v if k == keep else set()) for k, v in tabs.items()}
        return tabs

    bacc.get_activation_tables = patched
    bacc._act_tables_patched = True


def _build_graph(NS, meta):
    from concourse import library_config

    _patch_act_tables()
    nc = bacc.Bacc("TRN2", target_bir_lowering=False, debug=False,
                   num_devices=N_CORES)
    af = mybir.ActivationFunctionType

    xp = nc.declare_dram_parameter("x", [128, 4 * HALF], FP8, isOutput=False)
    wTp = nc.declare_dram_parameter("wT", [128, 4, NCHM], FP8, isOutput=False)
    biasp = nc.declare_dram_parameter("bias", [NCHP, 1], F32, isOutput=False)
    selp = nc.declare_dram_parameter("sel", [NCHP, NGRP], BF16, isOutput=False)
    gidxp = nc.declare_dram_parameter("gidx", [NCHP, NS // 16], mybir.dt.int16,
                                      isOutput=False)
    kmaskp = nc.declare_dram_parameter("kmask", [NCHP, NS], F32, isOutput=False)
    clabp = nc.declare_dram_parameter("clab", [NUM_CLS, NS], F32, isOutput=False)
    reglabp = nc.declare_dram_parameter("reglab", [CODE, NS], F32, isOutput=False)
    outp = nc.declare_dram_parameter("out", [1, 2], F32, isOutput=True)

    with tile.TileContext(nc) as tc:
        with (
            tc.tile_pool(name="const", bufs=1) as cpool,
            tc.tile_pool(name="xin", bufs=4) as xpool,
            tc.tile_pool(name="pred", bufs=1) as ppool,
            tc.tile_pool(name="post", bufs=1) as spool,
            tc.tile_pool(name="mmps", bufs=4, space=bass.MemorySpace.PSUM) as mps,
            tc.tile_pool(name="smps", bufs=1, space=bass.MemorySpace.PSUM) as sps,
        ):
            # load the ap_gather gpsimd ucode library up front, and fire a tiny
            # dummy gather: the Q7 cores fetch the ucode overlay lazily at the
            # first extended instruction (~17 us), so force that fetch early,
            # under the GEMM, instead of right before the real gather
            nc.gpsimd.load_library(library_config.ap_gather)
            dg_in = cpool.tile([16, 16], F32)
            nc.vector.memset(dg_in[:], 0.0)
            dg_idx = cpool.tile([16, 1], mybir.dt.int16)
            nc.vector.memset(dg_idx[:], 0)
            dg_out = cpool.tile([16, 16], F32)
            nc.gpsimd.ap_gather(dg_out[:], dg_in[:], dg_idx[:], channels=16,
                                num_elems=16, d=1, num_idxs=16)

            # activation-table prefetch: make the first ACT instruction an Exp
            # so the single covering table (ln/exp/relu/abs/square/copy) loads
            # once, early, overlapped with the first DMA slab.
            scratch = cpool.tile([1, 16], F32)
            nc.vector.memset(scratch[:], 0.0)
            nc.scalar.activation(scratch[:], scratch[:], af.Exp)

            # constants / per-core aux inputs on the ACT HWDGE ring so the x
            # slab DMAs on the sync ring are not queued behind them
            wt = cpool.tile([128, 4, NCHM], FP8)
            nc.scalar.dma_start(out=wt[:], in_=wTp[:])
            bias_t = cpool.tile([NCHP, 1], F32)
            nc.scalar.dma_start(out=bias_t[:], in_=biasp[:])
            sel_t = cpool.tile([NCHP, NGRP], BF16)
            nc.scalar.dma_start(out=sel_t[:], in_=selp[:])
            gidx_t = cpool.tile([NCHP, NS // 16], mybir.dt.int16)
            nc.scalar.dma_start(out=gidx_t[:], in_=gidxp[:])
            kmask_t = cpool.tile([NCHP, NS], F32)
            nc.scalar.dma_start(out=kmask_t[:], in_=kmaskp[:])
            clab_t = cpool.tile([NUM_CLS, NS], F32)
            nc.scalar.dma_start(out=clab_t[:], in_=clabp[:])
            reglab_t = cpool.tile([CODE, NS], F32)
            nc.scalar.dma_start(out=reglab_t[:], in_=reglabp[:])
            ones = cpool.tile([128, 1], BF16)
            nc.vector.memset(ones[:], 1.0)

            bsize = meta["bsize"]
            boffs = [0]
            for bs in bsize:
                boffs.append(boffs[-1] + bs)

            # single sample tiles; gathers fill G per bucket during the GEMM,
            # but nothing READS them until the end -- any mid-GEMM consumer
            # would head-of-line-block the evictions in the in-order DVE/ACT
            # queues whenever a gather runs late (Q7 cold-wake)
            G = spool.tile([NCHP, NS], F32)
            GM = spool.tile([NCHP, NS], BF16)

            nchunk = (NS + NTILE - 1) // NTILE
            # per-chunk accumulators (summed/combined at the very end)
            lse_parts = spool.tile([1, nchunk], F32)    # sum ln(sum exp(Y))
            pk_parts = spool.tile([NUM_CLS, nchunk], F32)  # sum clab*Y
            m_parts = spool.tile([CODE, nchunk], F32)   # sum (|d|-min(|d|,1))
            sq_parts = spool.tile([CODE, nchunk], F32)  # sum 0.5*min(|d|,1)^2
            onesf = cpool.tile([NCHM, 1], F32)
            nc.vector.memset(onesf[:], 1.0)
            z7 = cpool.tile([CODE, NTILE], F32)
            nc.vector.memset(z7[:], 0.0)

            # ---- dense 1x1-conv GEMM: pred = (W @ x)[:80]; evictions land in
            #      per-region pred tiles; a region's ap_gather fires once its
            #      last eviction lands ----
            gt_start = np.cumsum([0] + GTILES[:-1])
            gt_end = np.cumsum(GTILES)
            region_tiles = {}
            off = 0
            ti = 0
            for si, wdt in enumerate(SLABS):
                xt = xpool.tile([128, 4, wdt], FP8, tag="xt")
                # one contiguous DMA per slab (host pre-tilted the layout)
                nc.sync.dma_start(
                    out=xt[:],
                    in_=xp[:, 4 * off:4 * (off + wdt)].rearrange(
                        "p (k w) -> p k w", k=4))
                for t0 in range(0, wdt, NTILE):
                    tw = min(NTILE, wdt - t0)
                    gcol = off + t0
                    ri = int(np.searchsorted(gt_end, gcol, side="right"))
                    if ri not in region_tiles:
                        region_tiles[ri] = ppool.tile(
                            [NCHP, GTILES[ri]], F32, tag=f"pred{ri}",
                            name=f"pred{ri}")
                    pred_t = region_tiles[ri]
                    ps = mps.tile([NCHM, tw], F32, tag="mm")
                    for dd in range(2):
                        nc.tensor.matmul(
                            ps[:], wt[:, 2 * dd:2 * dd + 2, :],
                            xt[:, 2 * dd:2 * dd + 2, t0:t0 + tw],
                            start=(dd == 0), stop=(dd == 1),
                            perf_mode=mybir.MatmulPerfMode.DoubleRow)
                    rcol = gcol - int(gt_start[ri])
                    dst = pred_t[:, rcol:rcol + tw]
                    if ti % 2 == 0:
                        nc.vector.tensor_copy(dst, ps[0:NCHP, :])
                    else:
                        nc.scalar.copy(dst, ps[0:NCHP, :])
                    ti += 1
                    if gcol + tw == int(gt_end[ri]):
                        bs = bsize[ri]
                        bo = boffs[ri]
                        nc.gpsimd.ap_gather(
                            G[:, bo:bo + bs], pred_t[:],
                            gidx_t[:, bo // 16:(bo + bs) // 16],
                            channels=NCHP, num_elems=GTILES[ri], d=1,
                            num_idxs=bs)
                off += wdt

            # ---- post phase: bias+mask fuse, anchor-offset selection,
            #      softmax-CE and smooth-L1 partial sums ----
            # GM = (G + bias) * kmask, cast to bf16
            nc.vector.scalar_tensor_tensor(GM[:], G[:], bias_t[:, 0:1],
                                           kmask_t[:],
                                           op0=mybir.AluOpType.add,
                                           op1=mybir.AluOpType.mult)
            for ci, c0 in enumerate(range(0, NS, NTILE)):
                cw = min(NTILE, NS - c0)
                ch = slice(c0, c0 + cw)
                ycp = sps.tile([NUM_CLS, cw], F32, tag="yc")
                nc.tensor.matmul(ycp[:], sel_t[:, 0:NUM_CLS], GM[:, ch],
                                 start=True, stop=True)
                yrp = sps.tile([CODE, cw], F32, tag="yr")
                nc.tensor.matmul(yrp[:], sel_t[:, NUM_CLS:NGRP], GM[:, ch],
                                 start=True, stop=True)
                E = spool.tile([NUM_CLS, cw], BF16, tag=f"e{ci}", name=f"E{ci}")
                PKS = spool.tile([NUM_CLS, cw], F32, tag=f"p4{ci}",
                                 name=f"PKS{ci}")
                D = spool.tile([CODE, cw], F32, tag=f"d{ci}", name=f"D{ci}")
                nc.scalar.activation(E[:], ycp[:], af.Exp)
                # sum of picked logits: accumulate (Y * clab) per partition
                nc.vector.scalar_tensor_tensor(
                    PKS[:], ycp[:], 1.0, clab_t[:, ch],
                    op0=mybir.AluOpType.mult, op1=mybir.AluOpType.mult,
                    accum_out=pk_parts[:, ci:ci + 1])
                nc.vector.tensor_sub(D[:], yrp[:], reglab_t[:, ch])
                sep = sps.tile([1, cw], F32, tag="sp", name=f"sep{ci}")
                nc.tensor.matmul(sep[:], ones[0:NUM_CLS, :], E[:],
                                 start=True, stop=True)
                lse = spool.tile([1, cw], F32, tag=f"l{ci}", name=f"lse{ci}")
                nc.scalar.activation(lse[:], sep[:], af.Ln)
                lss = spool.tile([1, cw], F32, tag=f"ls{ci}", name=f"lss{ci}")
                nc.vector.scalar_tensor_tensor(
                    lss[:], lse[:], 0.0, z7[0:1, :cw],
                    op0=mybir.AluOpType.add, op1=mybir.AluOpType.add,
                    accum_out=lse_parts[:, ci:ci + 1])
                # smooth-L1 via m=min(|d|,1): sum 0.5*m^2 + sum(|d| - m)
                AD = spool.tile([CODE, cw], F32, tag=f"ad{ci}", name=f"AD{ci}")
                M1 = spool.tile([CODE, cw], F32, tag=f"m1{ci}", name=f"M1{ci}")
                T1 = spool.tile([CODE, cw], F32, tag=f"t1{ci}", name=f"T1{ci}")
                SQ = spool.tile([CODE, cw], F32, tag=f"sq{ci}", name=f"SQ{ci}")
                nc.scalar.activation(AD[:], D[:], af.Abs)
                nc.vector.tensor_scalar_min(M1[:], AD[:], 1.0)
                nc.vector.scalar_tensor_tensor(
                    T1[:], AD[:], 0.0, M1[:],
                    op0=mybir.AluOpType.add, op1=mybir.AluOpType.subtract,
                    accum_out=m_parts[:, ci:ci + 1])
                nc.vector.scalar_tensor_tensor(
                    SQ[:], M1[:], 0.5, M1[:],
                    op0=mybir.AluOpType.mult, op1=mybir.AluOpType.mult,
                    accum_out=sq_parts[:, ci:ci + 1])

            # ---- finals: cls = sum(lse) - sum(pk);
            #      reg = sum(0.5 m^2) + sum(|d| - m)  (m_parts holds |d|-m) ----
            regc = spool.tile([CODE, nchunk], F32)
            regrow = spool.tile([CODE, 1], F32)
            nc.vector.scalar_tensor_tensor(regc[:], sq_parts[:], 0.0,
                                           m_parts[:],
                                           op0=mybir.AluOpType.add,
                                           op1=mybir.AluOpType.add,
                                           accum_out=regrow[:])
            pkrow = spool.tile([NUM_CLS, 1], F32)
            nc.vector.reduce_sum(pkrow[:], pk_parts[:],
                                 axis=mybir.AxisListType.X)
            lse_sum = spool.tile([1, 1], F32)
            nc.vector.reduce_sum(lse_sum[:], lse_parts[:],
                                 axis=mybir.AxisListType.X)
            # cross-partition sums via tiny matmuls: [reg, pk] in one pass
            rsp = sps.tile([1, 1], F32, tag="sp", name="rsp")
            nc.tensor.matmul(rsp[:], onesf[0:CODE, :], regrow[:],
                             start=True, stop=True)
            pksp = sps.tile([1, 1], F32, tag="sp2", name="pksp")
            nc.tensor.matmul(pksp[:], onesf[0:NUM_CLS, :], pkrow[:],
                             start=True, stop=True)
            cls_sum = spool.tile([1, 1], F32)
            nc.vector.tensor_sub(cls_sum[:], lse_sum[:], pksp[:])

            outb = spool.tile([1, 2], F32)
            nc.scalar.copy(outb[0:1, 0:1], cls_sum[:])
            nc.scalar.copy(outb[0:1, 1:2], rsp[:])
            nc.sync.dma_start(out=outp[:], in_=outb[:])

    nc.compile()
    return nc


def kernel(**inputs):
    global LAST_RESULT
    if os.environ.get("BASS_LDW_OPT", "0") == "1":
        _patch_ldw_opt()
    if os.environ.get("BASS_FAST_TAIL", "1") == "1":
        _patch_fast_tail()
    in_maps, NS, meta = _host_prep(**inputs)
    nc = _build_graph(NS, meta)
    trace = os.environ.get("BASS_KERNEL_TRACE", "1") == "1"
    if trace:
        _ensure_ntff_hook()
    res = run_bass_kernel_spmd(nc, in_maps, list(range(N_CORES)), trace=trace)
    LAST_RESULT = res
    cls_sum = sum(float(r["out"][0, 0]) for r in res.results)
    reg_sum = sum(float(r["out"][0, 1]) for r in res.results)
    loss = CLS_W * cls_sum / N_SAMP + REG_W * reg_sum / (M_POS * CODE)
    return np.float32(loss)


# revision 61
# speedup vs baseline: 1.0430x; 1.0430x over previous
"""AnchorHeadBase forward+loss as a distributed Bass kernel on 8 TRN2 NeuronCores.

Reference computation:
  cls_pred = conv1x1(inputs, w_cls)  # [B, 24, H, W]
  reg_pred = conv1x1(inputs, w_reg)  # [B, 42, H, W]
  sample anchors at pos_ids/neg_ids, softmax-CE + smooth-L1 -> scalar loss.

Key identity: with pred = concat(cls_pred, reg_pred) viewed as [B, 66, HW],
the sampled value for channel-group g (cls class ci -> g=ci, reg code j ->
g=4+j) of anchor id a in batch b is pred[b, 6*g + a//HW, a % HW].

Sharding: data-parallel over batch + spatial halves: core = 2*b + (pos >= HW/2).
Each core runs the dense GEMM over its [512, 17600] shard (inputs/weights cast
to fp8e4m3 on host; f32 PSUM accumulation; output channels padded to 128 so the
compiler's fast-weight-load path triggers), keeps the [80, 17600] f32 pred map
in SBUF, gathers its sampled columns with gpsimd.ap_gather, selects the
per-sample anchor offset k with host-built 0/1 masks + a tiny selection matmul,
and computes softmax-CE / smooth-L1 partial sums on device. Host sums the 8
per-core partials (the unshard step) into the scalar loss.

Pad samples are neutralized without a validity mask: bias row 79 is 1 and
kmask[79, pad] = ln(1/4), so a pad sample's class logits are all ln(1/4),
logsumexp = 0, picked = 0; its reg rows are all 0 so smooth-L1 is 0.
"""

import os
import sys

sys.path.insert(0, "/opt/trn_rl_repo")

import numpy as np
import ml_dtypes

import concourse.bass as bass
import concourse.mybir as mybir
import concourse.tile as tile
from concourse import bacc
from concourse.bass_utils import run_bass_kernel_spmd


def _ensure_ntff_hook():
    """bass_utils' trace path needs antenv.axon_hooks; some containers lack the
    module (boot degrades silently and no hook gets registered). Install a
    holder module and register the ctypes-based NTFF hook ourselves."""
    import types

    try:
        from antenv import axon_hooks  # noqa: F401
    except ImportError:
        import antenv

        m = types.ModuleType("antenv.axon_hooks")
        m._hook = None

        def set_axon_ntff_profile_hook(h, _m=m):
            _m._hook = h

        def get_axon_ntff_profile_hook(_m=m):
            return _m._hook

        m.set_axon_ntff_profile_hook = set_axon_ntff_profile_hook
        m.get_axon_ntff_profile_hook = get_axon_ntff_profile_hook
        sys.modules["antenv.axon_hooks"] = m
        antenv.axon_hooks = m
    from antenv import axon_hooks
    if axon_hooks.get_axon_ntff_profile_hook() is None:
        try:
            from trn_agent_boot.trn_boot import _ntff_profile_via_ctypes
            so = "/opt/axon/libaxon_pjrt.so"
            if os.path.exists(so):
                axon_hooks.set_axon_ntff_profile_hook(_ntff_profile_via_ctypes(so))
        except Exception:
            pass

# ---- problem constants (hardcoded; must match the reference) ----
B, C_IN = 4, 512
NX, NY = 200, 176
HW = NX * NY              # 35200
HALF = HW // 2            # 17600 columns per core
N_CORES = 8
NUM_CLS, CODE, AGRP = 4, 7, 6
NCH = NUM_CLS * AGRP + CODE * AGRP   # 66 output channels (24 cls + 42 reg)
NCHP = 80                            # gather partition dim (%16 for ap_gather)
NCHM = 128                           # matmul M dim (128 -> fast weight load)
NGRP = NUM_CLS + CODE                # 11 channel groups
M_POS, M_NEG = 512, 4096
N_SAMP = M_NEG + M_POS               # 4608 cls samples (pos samples carry reg)
CLS_W, REG_W = 1.0, 2.0

# DMA slab widths: small first slabs so the PE starts early, then 4096-wide
# (512 KiB fp8 per k-chunk) steady-state slabs.
SLABS = [1024, 1024, 2048, 2048, 2048, 2048, 2048, 2048, 2048, 1216]
assert sum(SLABS) == HALF
# gather regions: early regions get their own pred tile + gather right after
# their evictions land (keeps the GPSIMD Q7s warm through the DMA-bound phase);
# every extended GPSIMD instruction pays ~2.4 us dispatch latency, so the tail
# of the map is ONE region with ONE gather instead of several trailing ones
GTILES = [1024, 2048, 2048, 2048, 2048, 4096, 4288]
assert sum(GTILES) == HALF
NTILE = 512                          # matmul moving free dim / PSUM bank

FP8 = mybir.dt.float8e4
FP8_NP = ml_dtypes.float8_e4m3
F32 = mybir.dt.float32
BF16 = mybir.dt.bfloat16

LAST_RESULT = None  # BassKernelResults of the most recent kernel() call


def _ceil_to(x, m):
    return (x + m - 1) // m * m


def _patch_fast_tail():
    """Trim the Tile end-of-kernel epilogue: skip the semaphore-clear pass and
    the second all-engine barrier. Safe for single-execution NEFFs (each
    kernel() call compiles, loads and runs the NEFF exactly once; the runtime
    zeroes semaphores at load)."""
    if getattr(tile.TileContext, "_fast_tail", False):
        return
    orig_dab = tile.TileContext._drain_and_barrier

    def fast(self, tick_clock, wait_clock):
        nc = self.nc
        orig_clear = nc.clear_and_free_semaphores
        orig_barrier = nc.all_engine_barrier
        calls = [0]

        def barrier_once(*a, **k):
            calls[0] += 1
            if calls[0] == 1:
                return orig_barrier(*a, **k)
            return None

        nc.clear_and_free_semaphores = lambda sems: None
        nc.all_engine_barrier = barrier_once
        try:
            orig_dab(self, tick_clock, wait_clock)
        finally:
            nc.clear_and_free_semaphores = orig_clear
            nc.all_engine_barrier = orig_barrier

    tile.TileContext._drain_and_barrier = fast
    tile.TileContext._fast_tail = True


def _patch_ldw_opt():
    """Flip walrus's --enable-ldw-opt on (dedupes back-to-back LDWEIGHTS of
    the same stationary operand)."""
    from concourse import bass_utils as bu
    if getattr(bu, "_ldw_patched", False):
        return
    orig = bu.run_command

    def patched(argv, **kw):
        argv = [a.replace("--enable-ldw-opt=false", "--enable-ldw-opt=true")
                if isinstance(a, str) else a for a in argv]
        return orig(argv, **kw)

    bu.run_command = patched
    bu._ldw_patched = True


def _host_prep(inputs, w_cls, b_cls, w_reg, b_reg, reg_labels, pos_ids, neg_ids,
               cls_labels):
    """Shard inputs, cast to fp8, and build per-core gather/mask tensors."""
    x = np.asarray(inputs, np.float32).reshape(B, C_IN, HW)
    W = np.concatenate([np.asarray(w_cls, np.float32),
                        np.asarray(w_reg, np.float32)], axis=0)     # [66, 512]
    bias = np.concatenate([np.asarray(b_cls, np.float32),
                           np.asarray(b_reg, np.float32)], axis=0)  # [66]
    W_pad = np.zeros((NCHM, C_IN), np.float32)
    W_pad[:NCH] = W
    bias_pad = np.zeros((NCHP, 1), np.float32)
    bias_pad[:NCH, 0] = bias
    bias_pad[79, 0] = 1.0            # pad-sample logsumexp neutralizer

    # lhsT layout: [128, 4, 128] fp8 -- wT[p, k, m] = W_pad[m, 128*k + p]
    wT = np.ascontiguousarray(
        W_pad.T.reshape(4, 128, NCHM).transpose(1, 0, 2)).astype(FP8_NP)

    # selection matrix (bf16): SEL[6*g + k, g] = 1; row 79 feeds the pad
    # neutralizer into every class logit.
    sel = np.zeros((NCHP, NGRP), np.float32)
    for g in range(NGRP):
        for k in range(AGRP):
            sel[6 * g + k, g] = 1.0
    sel[79, 0:NUM_CLS] = 1.0
    sel = sel.astype(ml_dtypes.bfloat16)

    # ---- partition the 4608 samples (neg first, then pos) by owning core ----
    pos_ids = np.asarray(pos_ids)
    neg_ids = np.asarray(neg_ids)
    cls_labels = np.asarray(cls_labels)
    reg_labels = np.asarray(reg_labels, np.float32)

    all_b = np.concatenate([neg_ids[:, 0], pos_ids[:, 0]]).astype(np.int64)
    all_a = np.concatenate([neg_ids[:, 1], pos_ids[:, 1]]).astype(np.int64)
    k_of = all_a // HW                      # anchor offset within pixel, 0..5
    pos_of = all_a % HW                     # spatial position
    core_of = 2 * all_b + (pos_of >= HALF)
    col_of = pos_of % HALF                  # column within the core's shard
    is_pos = np.arange(N_SAMP) >= M_NEG
    label = cls_labels.astype(np.int64)

    # samples are gathered per gather-tile (right after that tile's pred
    # columns land in SBUF), so bucket by (core, gtile)
    gt_off = np.cumsum([0] + GTILES[:-1])
    gt_id_of = np.searchsorted(np.cumsum(GTILES), col_of, side="right")

    NGT = len(GTILES)
    bucket_counts = np.zeros((N_CORES, NGT), np.int64)
    for c in range(N_CORES):
        for si in range(NGT):
            bucket_counts[c, si] = int(
                ((core_of == c) & (gt_id_of == si)).sum())
    # shared padded bucket sizes (same graph on every core); multiples of 32 so
    # every bucket's wrapped-index slice stays 4-byte aligned for the ucode's
    # 32-bit index reads
    bsize = [max(32, _ceil_to(int(bucket_counts[:, si].max()), 32))
             for si in range(NGT)]
    boff = np.cumsum([0] + bsize[:-1])
    NS = int(sum(bsize))
    meta = {"bsize": bsize}

    in_maps = []
    slab_offs = np.cumsum([0] + SLABS[:-1])
    for c in range(N_CORES):
        b_idx, half = c // 2, c % 2
        xs = np.ascontiguousarray(
            x[b_idx, :, half * HALF:(half + 1) * HALF]).astype(FP8_NP)
        # tilt per slab so each slab's [128, 4, w] SBUF tile is one contiguous
        # DRAM region (one dma_start with 16 KiB/partition descriptors)
        xt = np.empty((128, 4 * HALF), FP8_NP)
        for soff, w in zip(slab_offs, SLABS):
            for k in range(4):
                xt[:, 4 * soff + k * w:4 * soff + (k + 1) * w] = \
                    xs[128 * k:128 * (k + 1), soff:soff + w]

        # slot each sample into its gtile bucket (order within bucket is
        # arbitrary)
        jc = np.nonzero(core_of == c)[0]
        order = np.argsort(gt_id_of[jc], kind="stable")
        j = jc[order]
        sl = gt_id_of[j]
        s = np.zeros(len(j), np.int64)      # sample slot within [0, NS)
        for si in range(NGT):
            m = sl == si
            s[m] = boff[si] + np.arange(int(m.sum()))
        n = len(j)
        cols = col_of[j]
        ks = k_of[j]

        # gather indices are gtile-relative; wrapped per 16 partitions
        gidx16 = np.zeros((16, NS // 16), np.int16)
        gidx16[s % 16, s // 16] = (cols - gt_off[sl]).astype(np.int16)
        gidx = np.tile(gidx16, (NCHP // 16, 1))

        kmask = np.zeros((NCHP, NS), np.float32)
        for g in range(NUM_CLS):
            kmask[6 * g + ks, s] = 1.0
        jp = is_pos[j]
        sp = s[jp]
        for g in range(CODE):
            kmask[24 + 6 * g + ks[jp], sp] = 1.0
        pad = np.ones(NS, bool)
        pad[s] = False
        kmask[79, pad] = float(np.log(1.0 / NUM_CLS))  # pad neutralizer

        clab = np.zeros((NUM_CLS, NS), np.float32)
        clab[label[j], s] = 1.0

        reglab = np.zeros((CODE, NS), np.float32)
        reglab[:, sp] = reg_labels[j[jp] - M_NEG].T

        in_maps.append({
            "x": xt,
            "wT": wT,
            "bias": bias_pad,
            "sel": sel,
            "gidx": gidx,
            "kmask": kmask,
            "clab": clab,
            "reglab": reglab,
        })
    return in_maps, NS, meta


def _patch_act_tables():
    """Force the act-table pass to pick the one set covering every function we
    use (ln/exp/relu/abs/square/copy/identity) so exactly one table load is
    emitted, early. Ids are positional, so blank the other sets instead of
    filtering them out."""
    if getattr(bacc, "_act_tables_patched", False):
        return
    orig = bacc.get_activation_tables

    def patched(module_arch):
        tabs = dict(orig(module_arch))
        keep = "natural_log_exp_and_others"
        if keep in tabs:
            tabs = {k: (v if k == keep else set()) for k, v in tabs.items()}
        return tabs

    bacc.get_activation_tables = patched
    bacc._act_tables_patched = True


def _build_graph(NS, meta):
    from concourse import library_config

    _patch_act_tables()
    nc = bacc.Bacc("TRN2", target_bir_lowering=False, debug=False,
                   num_devices=N_CORES)
    af = mybir.ActivationFunctionType

    xp = nc.declare_dram_parameter("x", [128, 4 * HALF], FP8, isOutput=False)
    wTp = nc.declare_dram_parameter("wT", [128, 4, NCHM], FP8, isOutput=False)
    biasp = nc.declare_dram_parameter("bias", [NCHP, 1], F32, isOutput=False)
    selp = nc.declare_dram_parameter("sel", [NCHP, NGRP], BF16, isOutput=False)
    gidxp = nc.declare_dram_parameter("gidx", [NCHP, NS // 16], mybir.dt.int16,
                                      isOutput=False)
    kmaskp = nc.declare_dram_parameter("kmask", [NCHP, NS], F32, isOutput=False)
    clabp = nc.declare_dram_parameter("clab", [NUM_CLS, NS], F32, isOutput=False)
    reglabp = nc.declare_dram_parameter("reglab", [CODE, NS], F32, isOutput=False)
    outp = nc.declare_dram_parameter("out", [1, 2], F32, isOutput=True)

    with tile.TileContext(nc) as tc:
        with (
            tc.tile_pool(name="const", bufs=1) as cpool,
            tc.tile_pool(name="xin", bufs=4) as xpool,
            tc.tile_pool(name="pred", bufs=1) as ppool,
            tc.tile_pool(name="post", bufs=1) as spool,
            tc.tile_pool(name="mmps", bufs=4, space=bass.MemorySpace.PSUM) as mps,
            tc.tile_pool(name="smps", bufs=1, space=bass.MemorySpace.PSUM) as sps,
        ):
            # load the ap_gather gpsimd ucode library up front, and fire a tiny
            # dummy gather: the Q7 cores fetch the ucode overlay lazily at the
            # first extended instruction (~17 us), so force that fetch early,
            # under the GEMM, instead of right before the real gather
            nc.gpsimd.load_library(library_config.ap_gather)
            dg_in = cpool.tile([16, 16], F32)
            nc.vector.memset(dg_in[:], 0.0)
            dg_idx = cpool.tile([16, 1], mybir.dt.int16)
            nc.vector.memset(dg_idx[:], 0)
            dg_out = cpool.tile([16, 16], F32)
            nc.gpsimd.ap_gather(dg_out[:], dg_in[:], dg_idx[:], channels=16,
                                num_elems=16, d=1, num_idxs=16)

            # activation-table prefetch: make the first ACT instruction an Exp
            # so the single covering table (ln/exp/relu/abs/square/copy) loads
            # once, early, overlapped with the first DMA slab.
            scratch = cpool.tile([1, 16], F32)
            nc.vector.memset(scratch[:], 0.0)
            nc.scalar.activation(scratch[:], scratch[:], af.Exp)

            # constants / per-core aux inputs on the ACT HWDGE ring so the x
            # slab DMAs on the sync ring are not queued behind them
            wt = cpool.tile([128, 4, NCHM], FP8)
            nc.scalar.dma_start(out=wt[:], in_=wTp[:])
            bias_t = cpool.tile([NCHP, 1], F32)
            nc.scalar.dma_start(out=bias_t[:], in_=biasp[:])
            sel_t = cpool.tile([NCHP, NGRP], BF16)
            nc.scalar.dma_start(out=sel_t[:], in_=selp[:])
            gidx_t = cpool.tile([NCHP, NS // 16], mybir.dt.int16)
            nc.scalar.dma_start(out=gidx_t[:], in_=gidxp[:])
            kmask_t = cpool.tile([NCHP, NS], F32)
            nc.scalar.dma_start(out=kmask_t[:], in_=kmaskp[:])
            clab_t = cpool.tile([NUM_CLS, NS], F32)
            nc.scalar.dma_start(out=clab_t[:], in_=clabp[:])
            reglab_t = cpool.tile([CODE, NS], F32)
            nc.scalar.dma_start(out=reglab_t[:], in_=reglabp[:])
            ones = cpool.tile([128, 1], BF16)
            nc.vector.memset(ones[:], 1.0)

            bsize = meta["bsize"]
            boffs = [0]
            for bs in bsize:
                boffs.append(boffs[-1] + bs)

            # single sample tiles; gathers fill G per bucket during the GEMM,
            # but nothing READS them until the end -- any mid-GEMM consumer
            # would head-of-line-block the evictions in the in-order DVE/ACT
            # queues whenever a gather runs late (Q7 cold-wake)
            G = spool.tile([NCHP, NS], F32)
            GM = spool.tile([NCHP, NS], BF16)

            nchunk = (NS + NTILE - 1) // NTILE
            # per-chunk accumulators (summed/combined at the very end)
            lse_parts = spool.tile([1, nchunk], F32)    # sum ln(sum exp(Y))
            pk_parts = spool.tile([NUM_CLS, nchunk], F32)  # sum clab*Y
            m_parts = spool.tile([CODE, nchunk], F32)   # sum (|d|-min(|d|,1))
            sq_parts = spool.tile([CODE, nchunk], F32)  # sum 0.5*min(|d|,1)^2
            onesf = cpool.tile([NCHM, 1], F32)
            nc.vector.memset(onesf[:], 1.0)
            z7 = cpool.tile([CODE, NTILE], F32)
            nc.vector.memset(z7[:], 0.0)

            # ---- dense 1x1-conv GEMM: pred = (W @ x)[:80]; evictions land in
            #      per-region pred tiles; a region's ap_gather fires once its
            #      last eviction lands ----
            gt_start = np.cumsum([0] + GTILES[:-1])
            gt_end = np.cumsum(GTILES)
            region_tiles = {}
            off = 0
            ti = 0
            for si, wdt in enumerate(SLABS):
                xt = xpool.tile([128, 4, wdt], FP8, tag="xt")
                # one contiguous DMA per slab (host pre-tilted the layout)
                nc.sync.dma_start(
                    out=xt[:],
                    in_=xp[:, 4 * off:4 * (off + wdt)].rearrange(
                        "p (k w) -> p k w", k=4))
                for t0 in range(0, wdt, NTILE):
                    tw = min(NTILE, wdt - t0)
                    gcol = off + t0
                    ri = int(np.searchsorted(gt_end, gcol, side="right"))
                    if ri not in region_tiles:
                        region_tiles[ri] = ppool.tile(
                            [NCHP, GTILES[ri]], F32, tag=f"pred{ri}",
                            name=f"pred{ri}")
                    pred_t = region_tiles[ri]
                    ps = mps.tile([NCHM, tw], F32, tag="mm")
                    for dd in range(2):
                        nc.tensor.matmul(
                            ps[:], wt[:, 2 * dd:2 * dd + 2, :],
                            xt[:, 2 * dd:2 * dd + 2, t0:t0 + tw],
                            start=(dd == 0), stop=(dd == 1),
                            perf_mode=mybir.MatmulPerfMode.DoubleRow)
                    rcol = gcol - int(gt_start[ri])
                    dst = pred_t[:, rcol:rcol + tw]
                    if ti % 2 == 0:
                        nc.vector.tensor_copy(dst, ps[0:NCHP, :])
                    else:
                        nc.scalar.copy(dst, ps[0:NCHP, :])
                    ti += 1
                    if gcol + tw == int(gt_end[ri]):
                        bs = bsize[ri]
                        bo = boffs[ri]
                        nc.gpsimd.ap_gather(
                            G[:, bo:bo + bs], pred_t[:],
                            gidx_t[:, bo // 16:(bo + bs) // 16],
                            channels=NCHP, num_elems=GTILES[ri], d=1,
                            num_idxs=bs)
                off += wdt

            # ---- post phase: bias+mask fuse, anchor-offset selection,
            #      softmax-CE and smooth-L1 partial sums ----
            # GM = (G + bias) * kmask, cast to bf16
            nc.vector.scalar_tensor_tensor(GM[:], G[:], bias_t[:, 0:1],
                                           kmask_t[:],
                                           op0=mybir.AluOpType.add,
                                           op1=mybir.AluOpType.mult)
            for ci, c0 in enumerate(range(0, NS, NTILE)):
                cw = min(NTILE, NS - c0)
                ch = slice(c0, c0 + cw)
                ycp = sps.tile([NUM_CLS, cw], F32, tag="yc")
                nc.tensor.matmul(ycp[:], sel_t[:, 0:NUM_CLS], GM[:, ch],
                                 start=True, stop=True)
                yrp = sps.tile([CODE, cw], F32, tag="yr")
                nc.tensor.matmul(yrp[:], sel_t[:, NUM_CLS:NGRP], GM[:, ch],
                                 start=True, stop=True)
                E = spool.tile([NUM_CLS, cw], BF16, tag=f"e{ci}", name=f"E{ci}")
                PKS = spool.tile([NUM_CLS, cw], F32, tag=f"p4{ci}",
                                 name=f"PKS{ci}")
                D = spool.tile([CODE, cw], F32, tag=f"d{ci}", name=f"D{ci}")
                nc.scalar.activation(E[:], ycp[:], af.Exp)
                # sum of picked logits: accumulate (Y * clab) per partition
                nc.vector.scalar_tensor_tensor(
                    PKS[:], ycp[:], 1.0, clab_t[:, ch],
                    op0=mybir.AluOpType.mult, op1=mybir.AluOpType.mult,
                    accum_out=pk_parts[:, ci:ci + 1])
                nc.vector.tensor_sub(D[:], yrp[:], reglab_t[:, ch])
                sep = sps.tile([1, cw], F32, tag="sp", name=f"sep{ci}")
                nc.tensor.matmul(sep[:], ones[0:NUM_CLS, :], E[:],
                                 start=True, stop=True)
                lse = spool.tile([1, cw], F32, tag=f"l{ci}", name=f"lse{ci}")
                nc.scalar.activation(lse[:], sep[:], af.Ln)
                lss = spool.tile([1, cw], F32, tag=f"ls{ci}", name=f"lss{ci}")
                nc.vector.scalar_tensor_tensor(
                    lss[:], lse[:], 0.0, z7[0:1, :cw],
                    op0=mybir.AluOpType.add, op1=mybir.AluOpType.add,
                    accum_out=lse_parts[:, ci:ci + 1])
                # smooth-L1 via m=min(|d|,1): sum 0.5*m^2 + sum(|d| - m)
                AD = spool.tile([CODE, cw], F32, tag=f"ad{ci}", name=f"AD{ci}")
                M1 = spool.tile([CODE, cw], F32, tag=f"m1{ci}", name=f"M1{ci}")
                T1 = spool.tile([CODE, cw], F32, tag=f"t1{ci}", name=f"T1{ci}")
                SQ = spool.tile([CODE, cw], F32, tag=f"sq{ci}", name=f"SQ{ci}")
                nc.scalar.activation(AD[:], D[:], af.Abs)
                nc.vector.tensor_scalar_min(M1[:], AD[:], 1.0)
                nc.vector.scalar_tensor_tensor(
                    T1[:], AD[:], 0.0, M1[:],
                    op0=mybir.AluOpType.add, op1=mybir.AluOpType.subtract,
                    accum_out=m_parts[:, ci:ci + 1])
                nc.vector.scalar_tensor_tensor(
                    SQ[:], M1[:], 0.5, M1[:],
                    op0=mybir.AluOpType.mult, op1=mybir.AluOpType.mult,
                    accum_out=sq_parts[:, ci:ci + 1])

            # ---- finals: cls = sum(lse) - sum(pk);
            #      reg = sum(0.5 m^2) + sum(|d| - m)  (m_parts holds |d|-m) ----
            regc = spool.tile([CODE, nchunk], F32)
            regrow = spool.tile([CODE, 1], F32)
            nc.vector.scalar_tensor_tensor(regc[:], sq_parts[:], 0.0,
                                           m_parts[:],
                                           op0=mybir.AluOpType.add,
                                           op1=mybir.AluOpType.add,
                                           accum_out=regrow[:])
            pkrow = spool.tile([NUM_CLS, 1], F32)
            nc.vector.reduce_sum(pkrow[:], pk_parts[:],
                                 axis=mybir.AxisListType.X)
            lse_sum = spool.tile([1, 1], F32)
            nc.vector.reduce_sum(lse_sum[:], lse_parts[:],
                                 axis=mybir.AxisListType.X)
            # cross-partition sums via tiny matmuls: [reg, pk] in one pass
            rsp = sps.tile([1, 1], F32, tag="sp", name="rsp")
            nc.tensor.matmul(rsp[:], onesf[0:CODE, :], regrow[:],
                             start=True, stop=True)
            pksp = sps.tile([1, 1], F32, tag="sp2", name="pksp")
            nc.tensor.matmul(pksp[:], onesf[0:NUM_CLS, :], pkrow[:],
                             start=True, stop=True)
            cls_sum = spool.tile([1, 1], F32)
            nc.vector.tensor_sub(cls_sum[:], lse_sum[:], pksp[:])

            outb = spool.tile([1, 2], F32)
            nc.scalar.copy(outb[0:1, 0:1], cls_sum[:])
            nc.scalar.copy(outb[0:1, 1:2], rsp[:])
            nc.sync.dma_start(out=outp[:], in_=outb[:])

    nc.compile()
    return nc


def kernel(**inputs):
    global LAST_RESULT
    if os.environ.get("BASS_LDW_OPT", "0") == "1":
        _patch_ldw_opt()
    if os.environ.get("BASS_FAST_TAIL", "1") == "1":
        _patch_fast_tail()
    in_maps, NS, meta = _host_prep(**inputs)
    nc = _build_graph(NS, meta)
    trace = os.environ.get("BASS_KERNEL_TRACE", "1") == "1"
    if trace:
        _ensure_ntff_hook()
    res = run_bass_kernel_spmd(nc, in_maps, list(range(N_CORES)), trace=trace)
    LAST_RESULT = res
    cls_sum = sum(float(r["out"][0, 0]) for r in res.results)
    reg_sum = sum(float(r["out"][0, 1]) for r in res.results)
    loss = CLS_W * cls_sum / N_SAMP + REG_W * reg_sum / (M_POS * CODE)
    return np.float32(loss)


# revision 62
# speedup vs baseline: 1.0999x; 1.0546x over previous
"""AnchorHeadBase forward+loss as a distributed Bass kernel on 8 TRN2 NeuronCores.

Reference computation:
  cls_pred = conv1x1(inputs, w_cls)  # [B, 24, H, W]
  reg_pred = conv1x1(inputs, w_reg)  # [B, 42, H, W]
  sample anchors at pos_ids/neg_ids, softmax-CE + smooth-L1 -> scalar loss.

Key identity: with pred = concat(cls_pred, reg_pred) viewed as [B, 66, HW],
the sampled value for channel-group g (cls class ci -> g=ci, reg code j ->
g=4+j) of anchor id a in batch b is pred[b, 6*g + a//HW, a % HW].

Sharding: data-parallel over batch + spatial halves: core = 2*b + (pos >= HW/2).
Each core runs the dense GEMM over its [512, 17600] shard (inputs/weights cast
to fp8e4m3 on host; f32 PSUM accumulation; output channels padded to 128 so the
compiler's fast-weight-load path triggers), keeps the [80, 17600] f32 pred map
in SBUF, gathers its sampled columns with gpsimd.ap_gather, selects the
per-sample anchor offset k with host-built 0/1 masks + a tiny selection matmul,
and computes softmax-CE / smooth-L1 partial sums on device. Host sums the 8
per-core partials (the unshard step) into the scalar loss.

Pad samples are neutralized without a validity mask: bias row 79 is 1 and
kmask[79, pad] = ln(1/4), so a pad sample's class logits are all ln(1/4),
logsumexp = 0, picked = 0; its reg rows are all 0 so smooth-L1 is 0.
"""

import os
import sys

sys.path.insert(0, "/opt/trn_rl_repo")

import numpy as np
import ml_dtypes

import concourse.bass as bass
import concourse.mybir as mybir
import concourse.tile as tile
from concourse import bacc
from concourse.bass_utils import run_bass_kernel_spmd


def _ensure_ntff_hook():
    """bass_utils' trace path needs antenv.axon_hooks; some containers lack the
    module (boot degrades silently and no hook gets registered). Install a
    holder module and register the ctypes-based NTFF hook ourselves."""
    import types

    try:
        from antenv import axon_hooks  # noqa: F401
    except ImportError:
        import antenv

        m = types.ModuleType("antenv.axon_hooks")
        m._hook = None

        def set_axon_ntff_profile_hook(h, _m=m):
            _m._hook = h

        def get_axon_ntff_profile_hook(_m=m):
            return _m._hook

        m.set_axon_ntff_profile_hook = set_axon_ntff_profile_hook
        m.get_axon_ntff_profile_hook = get_axon_ntff_profile_hook
        sys.modules["antenv.axon_hooks"] = m
        antenv.axon_hooks = m
    from antenv import axon_hooks
    if axon_hooks.get_axon_ntff_profile_hook() is None:
        try:
            from trn_agent_boot.trn_boot import _ntff_profile_via_ctypes
            so = "/opt/axon/libaxon_pjrt.so"
            if os.path.exists(so):
                axon_hooks.set_axon_ntff_profile_hook(_ntff_profile_via_ctypes(so))
        except Exception:
            pass

# ---- problem constants (hardcoded; must match the reference) ----
B, C_IN = 4, 512
NX, NY = 200, 176
HW = NX * NY              # 35200
HALF = HW // 2            # 17600 columns per core
N_CORES = 8
NUM_CLS, CODE, AGRP = 4, 7, 6
NCH = NUM_CLS * AGRP + CODE * AGRP   # 66 output channels (24 cls + 42 reg)
NCHP = 80                            # gather partition dim (%16 for ap_gather)
NCHM = 128                           # matmul M dim (128 -> fast weight load)
NGRP = NUM_CLS + CODE                # 11 channel groups
M_POS, M_NEG = 512, 4096
N_SAMP = M_NEG + M_POS               # 4608 cls samples (pos samples carry reg)
CLS_W, REG_W = 1.0, 2.0

# DMA slab widths: small first slabs so the PE starts early, then 4096-wide
# (512 KiB fp8 per k-chunk) steady-state slabs.
SLABS = [1024, 1024, 2048, 2048, 2048, 2048, 2048, 2048, 2048, 1216]
assert sum(SLABS) == HALF
# gather regions: early regions get their own pred tile + gather right after
# their evictions land (keeps the GPSIMD Q7s warm through the DMA-bound phase);
# every extended GPSIMD instruction pays ~2.4 us dispatch latency, so the tail
# of the map is ONE region with ONE gather instead of several trailing ones
GTILES = [1024, 2048, 2048, 2048, 2048, 8384]
assert sum(GTILES) == HALF
NTILE = 512                          # matmul moving free dim / PSUM bank

FP8 = mybir.dt.float8e4
FP8_NP = ml_dtypes.float8_e4m3
F32 = mybir.dt.float32
BF16 = mybir.dt.bfloat16

LAST_RESULT = None  # BassKernelResults of the most recent kernel() call


def _ceil_to(x, m):
    return (x + m - 1) // m * m


def _patch_fast_tail():
    """Trim the Tile end-of-kernel epilogue: skip the semaphore-clear pass and
    the second all-engine barrier. Safe for single-execution NEFFs (each
    kernel() call compiles, loads and runs the NEFF exactly once; the runtime
    zeroes semaphores at load)."""
    if getattr(tile.TileContext, "_fast_tail", False):
        return
    orig_dab = tile.TileContext._drain_and_barrier

    def fast(self, tick_clock, wait_clock):
        nc = self.nc
        orig_clear = nc.clear_and_free_semaphores
        orig_barrier = nc.all_engine_barrier
        calls = [0]

        def barrier_once(*a, **k):
            calls[0] += 1
            if calls[0] == 1:
                return orig_barrier(*a, **k)
            return None

        nc.clear_and_free_semaphores = lambda sems: None
        nc.all_engine_barrier = barrier_once
        try:
            orig_dab(self, tick_clock, wait_clock)
        finally:
            nc.clear_and_free_semaphores = orig_clear
            nc.all_engine_barrier = orig_barrier

    tile.TileContext._drain_and_barrier = fast
    tile.TileContext._fast_tail = True


def _patch_ldw_opt():
    """Flip walrus's --enable-ldw-opt on (dedupes back-to-back LDWEIGHTS of
    the same stationary operand)."""
    from concourse import bass_utils as bu
    if getattr(bu, "_ldw_patched", False):
        return
    orig = bu.run_command

    def patched(argv, **kw):
        argv = [a.replace("--enable-ldw-opt=false", "--enable-ldw-opt=true")
                if isinstance(a, str) else a for a in argv]
        return orig(argv, **kw)

    bu.run_command = patched
    bu._ldw_patched = True


def _host_prep(inputs, w_cls, b_cls, w_reg, b_reg, reg_labels, pos_ids, neg_ids,
               cls_labels):
    """Shard inputs, cast to fp8, and build per-core gather/mask tensors."""
    x = np.asarray(inputs, np.float32).reshape(B, C_IN, HW)
    W = np.concatenate([np.asarray(w_cls, np.float32),
                        np.asarray(w_reg, np.float32)], axis=0)     # [66, 512]
    bias = np.concatenate([np.asarray(b_cls, np.float32),
                           np.asarray(b_reg, np.float32)], axis=0)  # [66]
    W_pad = np.zeros((NCHM, C_IN), np.float32)
    W_pad[:NCH] = W
    bias_pad = np.zeros((NCHP, 1), np.float32)
    bias_pad[:NCH, 0] = bias
    bias_pad[79, 0] = 1.0            # pad-sample logsumexp neutralizer

    # lhsT layout: [128, 4, 128] fp8 -- wT[p, k, m] = W_pad[m, 128*k + p]
    wT = np.ascontiguousarray(
        W_pad.T.reshape(4, 128, NCHM).transpose(1, 0, 2)).astype(FP8_NP)

    # selection matrix (bf16): SEL[6*g + k, g] = 1; row 79 feeds the pad
    # neutralizer into every class logit.
    sel = np.zeros((NCHP, NGRP), np.float32)
    for g in range(NGRP):
        for k in range(AGRP):
            sel[6 * g + k, g] = 1.0
    sel[79, 0:NUM_CLS] = 1.0
    sel = sel.astype(ml_dtypes.bfloat16)

    # ---- partition the 4608 samples (neg first, then pos) by owning core ----
    pos_ids = np.asarray(pos_ids)
    neg_ids = np.asarray(neg_ids)
    cls_labels = np.asarray(cls_labels)
    reg_labels = np.asarray(reg_labels, np.float32)

    all_b = np.concatenate([neg_ids[:, 0], pos_ids[:, 0]]).astype(np.int64)
    all_a = np.concatenate([neg_ids[:, 1], pos_ids[:, 1]]).astype(np.int64)
    k_of = all_a // HW                      # anchor offset within pixel, 0..5
    pos_of = all_a % HW                     # spatial position
    core_of = 2 * all_b + (pos_of >= HALF)
    col_of = pos_of % HALF                  # column within the core's shard
    is_pos = np.arange(N_SAMP) >= M_NEG
    label = cls_labels.astype(np.int64)

    # samples are gathered per gather-tile (right after that tile's pred
    # columns land in SBUF), so bucket by (core, gtile)
    gt_off = np.cumsum([0] + GTILES[:-1])
    gt_id_of = np.searchsorted(np.cumsum(GTILES), col_of, side="right")

    NGT = len(GTILES)
    bucket_counts = np.zeros((N_CORES, NGT), np.int64)
    for c in range(N_CORES):
        for si in range(NGT):
            bucket_counts[c, si] = int(
                ((core_of == c) & (gt_id_of == si)).sum())
    # shared padded bucket sizes (same graph on every core); multiples of 32 so
    # every bucket's wrapped-index slice stays 4-byte aligned for the ucode's
    # 32-bit index reads
    bsize = [max(32, _ceil_to(int(bucket_counts[:, si].max()), 32))
             for si in range(NGT)]
    boff = np.cumsum([0] + bsize[:-1])
    NS = int(sum(bsize))
    meta = {"bsize": bsize}

    in_maps = []
    slab_offs = np.cumsum([0] + SLABS[:-1])
    for c in range(N_CORES):
        b_idx, half = c // 2, c % 2
        xs = np.ascontiguousarray(
            x[b_idx, :, half * HALF:(half + 1) * HALF]).astype(FP8_NP)
        # tilt per slab so each slab's [128, 4, w] SBUF tile is one contiguous
        # DRAM region (one dma_start with 16 KiB/partition descriptors)
        xt = np.empty((128, 4 * HALF), FP8_NP)
        for soff, w in zip(slab_offs, SLABS):
            for k in range(4):
                xt[:, 4 * soff + k * w:4 * soff + (k + 1) * w] = \
                    xs[128 * k:128 * (k + 1), soff:soff + w]

        # slot each sample into its gtile bucket (order within bucket is
        # arbitrary)
        jc = np.nonzero(core_of == c)[0]
        order = np.argsort(gt_id_of[jc], kind="stable")
        j = jc[order]
        sl = gt_id_of[j]
        s = np.zeros(len(j), np.int64)      # sample slot within [0, NS)
        for si in range(NGT):
            m = sl == si
            s[m] = boff[si] + np.arange(int(m.sum()))
        n = len(j)
        cols = col_of[j]
        ks = k_of[j]

        # gather indices are gtile-relative; wrapped per 16 partitions
        gidx16 = np.zeros((16, NS // 16), np.int16)
        gidx16[s % 16, s // 16] = (cols - gt_off[sl]).astype(np.int16)
        gidx = np.tile(gidx16, (NCHP // 16, 1))

        kmask = np.zeros((NCHP, NS), np.float32)
        for g in range(NUM_CLS):
            kmask[6 * g + ks, s] = 1.0
        jp = is_pos[j]
        sp = s[jp]
        for g in range(CODE):
            kmask[24 + 6 * g + ks[jp], sp] = 1.0
        pad = np.ones(NS, bool)
        pad[s] = False
        kmask[79, pad] = float(np.log(1.0 / NUM_CLS))  # pad neutralizer

        clab = np.zeros((NUM_CLS, NS), np.float32)
        clab[label[j], s] = 1.0

        reglab = np.zeros((CODE, NS), np.float32)
        reglab[:, sp] = reg_labels[j[jp] - M_NEG].T

        in_maps.append({
            "x": xt,
            "wT": wT,
            "bias": bias_pad,
            "sel": sel,
            "gidx": gidx,
            "kmask": kmask,
            "clab": clab,
            "reglab": reglab,
        })
    return in_maps, NS, meta


def _patch_act_tables():
    """Force the act-table pass to pick the one set covering every function we
    use (ln/exp/relu/abs/square/copy/identity) so exactly one table load is
    emitted, early. Ids are positional, so blank the other sets instead of
    filtering them out."""
    if getattr(bacc, "_act_tables_patched", False):
        return
    orig = bacc.get_activation_tables

    def patched(module_arch):
        tabs = dict(orig(module_arch))
        keep = "natural_log_exp_and_others"
        if keep in tabs:
            tabs = {k: (v if k == keep else set()) for k, v in tabs.items()}
        return tabs

    bacc.get_activation_tables = patched
    bacc._act_tables_patched = True


def _build_graph(NS, meta):
    from concourse import library_config

    _patch_act_tables()
    nc = bacc.Bacc("TRN2", target_bir_lowering=False, debug=False,
                   num_devices=N_CORES)
    af = mybir.ActivationFunctionType

    xp = nc.declare_dram_parameter("x", [128, 4 * HALF], FP8, isOutput=False)
    wTp = nc.declare_dram_parameter("wT", [128, 4, NCHM], FP8, isOutput=False)
    biasp = nc.declare_dram_parameter("bias", [NCHP, 1], F32, isOutput=False)
    selp = nc.declare_dram_parameter("sel", [NCHP, NGRP], BF16, isOutput=False)
    gidxp = nc.declare_dram_parameter("gidx", [NCHP, NS // 16], mybir.dt.int16,
                                      isOutput=False)
    kmaskp = nc.declare_dram_parameter("kmask", [NCHP, NS], F32, isOutput=False)
    clabp = nc.declare_dram_parameter("clab", [NUM_CLS, NS], F32, isOutput=False)
    reglabp = nc.declare_dram_parameter("reglab", [CODE, NS], F32, isOutput=False)
    outp = nc.declare_dram_parameter("out", [1, 2], F32, isOutput=True)

    with tile.TileContext(nc) as tc:
        with (
            tc.tile_pool(name="const", bufs=1) as cpool,
            tc.tile_pool(name="xin", bufs=4) as xpool,
            tc.tile_pool(name="pred", bufs=1) as ppool,
            tc.tile_pool(name="post", bufs=1) as spool,
            tc.tile_pool(name="mmps", bufs=4, space=bass.MemorySpace.PSUM) as mps,
            tc.tile_pool(name="smps", bufs=1, space=bass.MemorySpace.PSUM) as sps,
        ):
            # load the ap_gather gpsimd ucode library up front, and fire a tiny
            # dummy gather: the Q7 cores fetch the ucode overlay lazily at the
            # first extended instruction (~17 us), so force that fetch early,
            # under the GEMM, instead of right before the real gather
            nc.gpsimd.load_library(library_config.ap_gather)
            dg_in = cpool.tile([16, 16], F32)
            nc.vector.memset(dg_in[:], 0.0)
            dg_idx = cpool.tile([16, 1], mybir.dt.int16)
            nc.vector.memset(dg_idx[:], 0)
            dg_out = cpool.tile([16, 16], F32)
            nc.gpsimd.ap_gather(dg_out[:], dg_in[:], dg_idx[:], channels=16,
                                num_elems=16, d=1, num_idxs=16)

            # activation-table prefetch: make the first ACT instruction an Exp
            # so the single covering table (ln/exp/relu/abs/square/copy) loads
            # once, early, overlapped with the first DMA slab.
            scratch = cpool.tile([1, 16], F32)
            nc.vector.memset(scratch[:], 0.0)
            nc.scalar.activation(scratch[:], scratch[:], af.Exp)

            # constants / per-core aux inputs on the ACT HWDGE ring so the x
            # slab DMAs on the sync ring are not queued behind them
            wt = cpool.tile([128, 4, NCHM], FP8)
            nc.scalar.dma_start(out=wt[:], in_=wTp[:])
            bias_t = cpool.tile([NCHP, 1], F32)
            nc.scalar.dma_start(out=bias_t[:], in_=biasp[:])
            sel_t = cpool.tile([NCHP, NGRP], BF16)
            nc.scalar.dma_start(out=sel_t[:], in_=selp[:])
            gidx_t = cpool.tile([NCHP, NS // 16], mybir.dt.int16)
            nc.scalar.dma_start(out=gidx_t[:], in_=gidxp[:])
            kmask_t = cpool.tile([NCHP, NS], F32)
            nc.scalar.dma_start(out=kmask_t[:], in_=kmaskp[:])
            clab_t = cpool.tile([NUM_CLS, NS], F32)
            nc.scalar.dma_start(out=clab_t[:], in_=clabp[:])
            reglab_t = cpool.tile([CODE, NS], F32)
            nc.scalar.dma_start(out=reglab_t[:], in_=reglabp[:])
            ones = cpool.tile([128, 1], BF16)
            nc.vector.memset(ones[:], 1.0)

            bsize = meta["bsize"]
            boffs = [0]
            for bs in bsize:
                boffs.append(boffs[-1] + bs)

            # single sample tiles; gathers fill G per bucket during the GEMM,
            # but nothing READS them until the end -- any mid-GEMM consumer
            # would head-of-line-block the evictions in the in-order DVE/ACT
            # queues whenever a gather runs late (Q7 cold-wake)
            G = spool.tile([NCHP, NS], F32)
            GM = spool.tile([NCHP, NS], BF16)

            nchunk = (NS + NTILE - 1) // NTILE
            # per-chunk accumulators (summed/combined at the very end)
            lse_parts = spool.tile([1, nchunk], F32)    # sum ln(sum exp(Y))
            pk_parts = spool.tile([NUM_CLS, nchunk], F32)  # sum clab*Y
            m_parts = spool.tile([CODE, nchunk], F32)   # sum (|d|-min(|d|,1))
            sq_parts = spool.tile([CODE, nchunk], F32)  # sum 0.5*min(|d|,1)^2
            onesf = cpool.tile([NCHM, 1], F32)
            nc.vector.memset(onesf[:], 1.0)
            z7 = cpool.tile([CODE, NTILE], F32)
            nc.vector.memset(z7[:], 0.0)

            # ---- dense 1x1-conv GEMM: pred = (W @ x)[:80]; evictions land in
            #      per-region pred tiles; a region's ap_gather fires once its
            #      last eviction lands ----
            gt_start = np.cumsum([0] + GTILES[:-1])
            gt_end = np.cumsum(GTILES)
            region_tiles = {}
            off = 0
            ti = 0
            for si, wdt in enumerate(SLABS):
                xt = xpool.tile([128, 4, wdt], FP8, tag="xt")
                # one contiguous DMA per slab (host pre-tilted the layout)
                nc.sync.dma_start(
                    out=xt[:],
                    in_=xp[:, 4 * off:4 * (off + wdt)].rearrange(
                        "p (k w) -> p k w", k=4))
                for t0 in range(0, wdt, NTILE):
                    tw = min(NTILE, wdt - t0)
                    gcol = off + t0
                    ri = int(np.searchsorted(gt_end, gcol, side="right"))
                    if ri not in region_tiles:
                        region_tiles[ri] = ppool.tile(
                            [NCHP, GTILES[ri]], F32, tag=f"pred{ri}",
                            name=f"pred{ri}")
                    pred_t = region_tiles[ri]
                    ps = mps.tile([NCHM, tw], F32, tag="mm")
                    for dd in range(2):
                        nc.tensor.matmul(
                            ps[:], wt[:, 2 * dd:2 * dd + 2, :],
                            xt[:, 2 * dd:2 * dd + 2, t0:t0 + tw],
                            start=(dd == 0), stop=(dd == 1),
                            perf_mode=mybir.MatmulPerfMode.DoubleRow)
                    rcol = gcol - int(gt_start[ri])
                    dst = pred_t[:, rcol:rcol + tw]
                    if ti % 2 == 0:
                        nc.vector.tensor_copy(dst, ps[0:NCHP, :])
                    else:
                        nc.scalar.copy(dst, ps[0:NCHP, :])
                    ti += 1
                    if gcol + tw == int(gt_end[ri]):
                        bs = bsize[ri]
                        bo = boffs[ri]
                        nc.gpsimd.ap_gather(
                            G[:, bo:bo + bs], pred_t[:],
                            gidx_t[:, bo // 16:(bo + bs) // 16],
                            channels=NCHP, num_elems=GTILES[ri], d=1,
                            num_idxs=bs)
                off += wdt

            # keep the PE's HAM clock warm across the trailing-gather wait so
            # the post-phase matmuls run at full clock (dead writes, no reader)
            for wi in range(16):
                wps = mps.tile([NCHM, NTILE], F32, tag="mm", name=f"warm{wi}")
                nc.tensor.matmul(wps[:], wt[:, 0:2, :], xt[:, 0:2, 0:NTILE],
                                 start=True, stop=True,
                                 perf_mode=mybir.MatmulPerfMode.DoubleRow)

            # ---- post phase: bias+mask fuse, anchor-offset selection,
            #      softmax-CE and smooth-L1 partial sums ----
            # GM = (G + bias) * kmask, cast to bf16
            nc.vector.scalar_tensor_tensor(GM[:], G[:], bias_t[:, 0:1],
                                           kmask_t[:],
                                           op0=mybir.AluOpType.add,
                                           op1=mybir.AluOpType.mult)
            for ci, c0 in enumerate(range(0, NS, NTILE)):
                cw = min(NTILE, NS - c0)
                ch = slice(c0, c0 + cw)
                ycp = sps.tile([NUM_CLS, cw], F32, tag="yc")
                nc.tensor.matmul(ycp[:], sel_t[:, 0:NUM_CLS], GM[:, ch],
                                 start=True, stop=True)
                yrp = sps.tile([CODE, cw], F32, tag="yr")
                nc.tensor.matmul(yrp[:], sel_t[:, NUM_CLS:NGRP], GM[:, ch],
                                 start=True, stop=True)
                E = spool.tile([NUM_CLS, cw], BF16, tag=f"e{ci}", name=f"E{ci}")
                PKS = spool.tile([NUM_CLS, cw], F32, tag=f"p4{ci}",
                                 name=f"PKS{ci}")
                D = spool.tile([CODE, cw], F32, tag=f"d{ci}", name=f"D{ci}")
                nc.scalar.activation(E[:], ycp[:], af.Exp)
                # sum of picked logits: accumulate (Y * clab) per partition
                nc.vector.scalar_tensor_tensor(
                    PKS[:], ycp[:], 1.0, clab_t[:, ch],
                    op0=mybir.AluOpType.mult, op1=mybir.AluOpType.mult,
                    accum_out=pk_parts[:, ci:ci + 1])
                nc.vector.tensor_sub(D[:], yrp[:], reglab_t[:, ch])
                sep = sps.tile([1, cw], F32, tag="sp", name=f"sep{ci}")
                nc.tensor.matmul(sep[:], ones[0:NUM_CLS, :], E[:],
                                 start=True, stop=True)
                lse = spool.tile([1, cw], F32, tag=f"l{ci}", name=f"lse{ci}")
                nc.scalar.activation(lse[:], sep[:], af.Ln)
                lss = spool.tile([1, cw], F32, tag=f"ls{ci}", name=f"lss{ci}")
                nc.vector.scalar_tensor_tensor(
                    lss[:], lse[:], 0.0, z7[0:1, :cw],
                    op0=mybir.AluOpType.add, op1=mybir.AluOpType.add,
                    accum_out=lse_parts[:, ci:ci + 1])
                # smooth-L1 via m=min(|d|,1): sum 0.5*m^2 + sum(|d| - m)
                AD = spool.tile([CODE, cw], F32, tag=f"ad{ci}", name=f"AD{ci}")
                M1 = spool.tile([CODE, cw], F32, tag=f"m1{ci}", name=f"M1{ci}")
                T1 = spool.tile([CODE, cw], F32, tag=f"t1{ci}", name=f"T1{ci}")
                SQ = spool.tile([CODE, cw], F32, tag=f"sq{ci}", name=f"SQ{ci}")
                nc.scalar.activation(AD[:], D[:], af.Abs)
                nc.vector.tensor_scalar_min(M1[:], AD[:], 1.0)
                nc.vector.scalar_tensor_tensor(
                    T1[:], AD[:], 0.0, M1[:],
                    op0=mybir.AluOpType.add, op1=mybir.AluOpType.subtract,
                    accum_out=m_parts[:, ci:ci + 1])
                nc.vector.scalar_tensor_tensor(
                    SQ[:], M1[:], 0.5, M1[:],
                    op0=mybir.AluOpType.mult, op1=mybir.AluOpType.mult,
                    accum_out=sq_parts[:, ci:ci + 1])

            # ---- finals: cls = sum(lse) - sum(pk);
            #      reg = sum(0.5 m^2) + sum(|d| - m)  (m_parts holds |d|-m) ----
            regc = spool.tile([CODE, nchunk], F32)
            regrow = spool.tile([CODE, 1], F32)
            nc.vector.scalar_tensor_tensor(regc[:], sq_parts[:], 0.0,
                                           m_parts[:],
                                           op0=mybir.AluOpType.add,
                                           op1=mybir.AluOpType.add,
                                           accum_out=regrow[:])
            pkrow = spool.tile([NUM_CLS, 1], F32)
            nc.vector.reduce_sum(pkrow[:], pk_parts[:],
                                 axis=mybir.AxisListType.X)
            lse_sum = spool.tile([1, 1], F32)
            nc.vector.reduce_sum(lse_sum[:], lse_parts[:],
                                 axis=mybir.AxisListType.X)
            # cross-partition sums via tiny matmuls: [reg, pk] in one pass
            rsp = sps.tile([1, 1], F32, tag="sp", name="rsp")
            nc.tensor.matmul(rsp[:], onesf[0:CODE, :], regrow[:],
                             start=True, stop=True)
            pksp = sps.tile([1, 1], F32, tag="sp2", name="pksp")
            nc.tensor.matmul(pksp[:], onesf[0:NUM_CLS, :], pkrow[:],
                             start=True, stop=True)
            cls_sum = spool.tile([1, 1], F32)
            nc.vector.tensor_sub(cls_sum[:], lse_sum[:], pksp[:])

            outb = spool.tile([1, 2], F32)
            nc.scalar.copy(outb[0:1, 0:1], cls_sum[:])
            nc.scalar.copy(outb[0:1, 1:2], rsp[:])
            nc.sync.dma_start(out=outp[:], in_=outb[:])

    nc.compile()
    return nc


def kernel(**inputs):
    global LAST_RESULT
    if os.environ.get("BASS_LDW_OPT", "0") == "1":
        _patch_ldw_opt()
    if os.environ.get("BASS_FAST_TAIL", "1") == "1":
        _patch_fast_tail()
    in_maps, NS, meta = _host_prep(**inputs)
    nc = _build_graph(NS, meta)
    trace = os.environ.get("BASS_KERNEL_TRACE", "1") == "1"
    if trace:
        _ensure_ntff_hook()
    res = run_bass_kernel_spmd(nc, in_maps, list(range(N_CORES)), trace=trace)
    LAST_RESULT = res
    cls_sum = sum(float(r["out"][0, 0]) for r in res.results)
    reg_sum = sum(float(r["out"][0, 1]) for r in res.results)
    loss = CLS_W * cls_sum / N_SAMP + REG_W * reg_sum / (M_POS * CODE)
    return np.float32(loss)


# revision 63
# speedup vs baseline: 1.1387x; 1.0352x over previous
"""AnchorHeadBase forward+loss as a distributed Bass kernel on 8 TRN2 NeuronCores.

Reference computation:
  cls_pred = conv1x1(inputs, w_cls)  # [B, 24, H, W]
  reg_pred = conv1x1(inputs, w_reg)  # [B, 42, H, W]
  sample anchors at pos_ids/neg_ids, softmax-CE + smooth-L1 -> scalar loss.

Key identity: with pred = concat(cls_pred, reg_pred) viewed as [B, 66, HW],
the sampled value for channel-group g (cls class ci -> g=ci, reg code j ->
g=4+j) of anchor id a in batch b is pred[b, 6*g + a//HW, a % HW].

Sharding: data-parallel over batch + spatial halves: core = 2*b + (pos >= HW/2).
Each core runs the dense GEMM over its [512, 17600] shard (inputs/weights cast
to fp8e4m3 on host; f32 PSUM accumulation; output channels padded to 128 so the
compiler's fast-weight-load path triggers), keeps the [80, 17600] f32 pred map
in SBUF, gathers its sampled columns with gpsimd.ap_gather, selects the
per-sample anchor offset k with host-built 0/1 masks + a tiny selection matmul,
and computes softmax-CE / smooth-L1 partial sums on device. Host sums the 8
per-core partials (the unshard step) into the scalar loss.

Pad samples are neutralized without a validity mask: bias row 79 is 1 and
kmask[79, pad] = ln(1/4), so a pad sample's class logits are all ln(1/4),
logsumexp = 0, picked = 0; its reg rows are all 0 so smooth-L1 is 0.
"""

import os
import sys

sys.path.insert(0, "/opt/trn_rl_repo")

import numpy as np
import ml_dtypes

import concourse.bass as bass
import concourse.mybir as mybir
import concourse.tile as tile
from concourse import bacc
from concourse.bass_utils import run_bass_kernel_spmd


def _ensure_ntff_hook():
    """bass_utils' trace path needs antenv.axon_hooks; some containers lack the
    module (boot degrades silently and no hook gets registered). Install a
    holder module and register the ctypes-based NTFF hook ourselves."""
    import types

    try:
        from antenv import axon_hooks  # noqa: F401
    except ImportError:
        import antenv

        m = types.ModuleType("antenv.axon_hooks")
        m._hook = None

        def set_axon_ntff_profile_hook(h, _m=m):
            _m._hook = h

        def get_axon_ntff_profile_hook(_m=m):
            return _m._hook

        m.set_axon_ntff_profile_hook = set_axon_ntff_profile_hook
        m.get_axon_ntff_profile_hook = get_axon_ntff_profile_hook
        sys.modules["antenv.axon_hooks"] = m
        antenv.axon_hooks = m
    from antenv import axon_hooks
    if axon_hooks.get_axon_ntff_profile_hook() is None:
        try:
            from trn_agent_boot.trn_boot import _ntff_profile_via_ctypes
            so = "/opt/axon/libaxon_pjrt.so"
            if os.path.exists(so):
                axon_hooks.set_axon_ntff_profile_hook(_ntff_profile_via_ctypes(so))
        except Exception:
            pass

# ---- problem constants (hardcoded; must match the reference) ----
B, C_IN = 4, 512
NX, NY = 200, 176
HW = NX * NY              # 35200
HALF = HW // 2            # 17600 columns per core
N_CORES = 8
NUM_CLS, CODE, AGRP = 4, 7, 6
NCH = NUM_CLS * AGRP + CODE * AGRP   # 66 output channels (24 cls + 42 reg)
NCHP = 80                            # gather partition dim (%16 for ap_gather)
NCHM = 128                           # matmul M dim (128 -> fast weight load)
NGRP = NUM_CLS + CODE                # 11 channel groups
M_POS, M_NEG = 512, 4096
N_SAMP = M_NEG + M_POS               # 4608 cls samples (pos samples carry reg)
CLS_W, REG_W = 1.0, 2.0

# DMA slab widths: small first slabs so the PE starts early, then 4096-wide
# (512 KiB fp8 per k-chunk) steady-state slabs.
SLABS = [1024, 1024, 2048, 2048, 2048, 2048, 2048, 2048, 2048, 1216]
assert sum(SLABS) == HALF
# gather regions: early regions get their own pred tile + gather right after
# their evictions land (keeps the GPSIMD Q7s warm through the DMA-bound phase);
# every extended GPSIMD instruction pays ~2.4 us dispatch latency, so the tail
# of the map is ONE region with ONE gather instead of several trailing ones
GTILES = [1024, 2048, 2048, 2048, 2048, 8384]
assert sum(GTILES) == HALF
NTILE = 512                          # matmul moving free dim / PSUM bank

FP8 = mybir.dt.float8e4
FP8_NP = ml_dtypes.float8_e4m3
F32 = mybir.dt.float32
BF16 = mybir.dt.bfloat16

LAST_RESULT = None  # BassKernelResults of the most recent kernel() call


def _ceil_to(x, m):
    return (x + m - 1) // m * m


def _patch_fast_tail():
    """Trim the Tile end-of-kernel epilogue: skip the semaphore-clear pass and
    the second all-engine barrier. Safe for single-execution NEFFs (each
    kernel() call compiles, loads and runs the NEFF exactly once; the runtime
    zeroes semaphores at load)."""
    if getattr(tile.TileContext, "_fast_tail", False):
        return
    orig_dab = tile.TileContext._drain_and_barrier

    def fast(self, tick_clock, wait_clock):
        nc = self.nc
        orig_clear = nc.clear_and_free_semaphores
        orig_barrier = nc.all_engine_barrier
        calls = [0]

        def barrier_once(*a, **k):
            calls[0] += 1
            if calls[0] == 1:
                return orig_barrier(*a, **k)
            return None

        nc.clear_and_free_semaphores = lambda sems: None
        nc.all_engine_barrier = barrier_once
        try:
            orig_dab(self, tick_clock, wait_clock)
        finally:
            nc.clear_and_free_semaphores = orig_clear
            nc.all_engine_barrier = orig_barrier

    tile.TileContext._drain_and_barrier = fast
    tile.TileContext._fast_tail = True


def _patch_ldw_opt():
    """Flip walrus's --enable-ldw-opt on (dedupes back-to-back LDWEIGHTS of
    the same stationary operand)."""
    from concourse import bass_utils as bu
    if getattr(bu, "_ldw_patched", False):
        return
    orig = bu.run_command

    def patched(argv, **kw):
        argv = [a.replace("--enable-ldw-opt=false", "--enable-ldw-opt=true")
                if isinstance(a, str) else a for a in argv]
        return orig(argv, **kw)

    bu.run_command = patched
    bu._ldw_patched = True


def _host_prep(inputs, w_cls, b_cls, w_reg, b_reg, reg_labels, pos_ids, neg_ids,
               cls_labels):
    """Shard inputs, cast to fp8, and build per-core gather/mask tensors."""
    x = np.asarray(inputs, np.float32).reshape(B, C_IN, HW)
    W = np.concatenate([np.asarray(w_cls, np.float32),
                        np.asarray(w_reg, np.float32)], axis=0)     # [66, 512]
    bias = np.concatenate([np.asarray(b_cls, np.float32),
                           np.asarray(b_reg, np.float32)], axis=0)  # [66]
    W_pad = np.zeros((NCHM, C_IN), np.float32)
    W_pad[:NCH] = W
    bias_pad = np.zeros((NCHP, 1), np.float32)
    bias_pad[:NCH, 0] = bias
    bias_pad[79, 0] = 1.0            # pad-sample logsumexp neutralizer

    # lhsT layout: [128, 4, 128] fp8 -- wT[p, k, m] = W_pad[m, 128*k + p]
    wT = np.ascontiguousarray(
        W_pad.T.reshape(4, 128, NCHM).transpose(1, 0, 2)).astype(FP8_NP)

    # selection matrix (bf16): SEL[6*g + k, g] = 1; row 79 feeds the pad
    # neutralizer into every class logit.
    sel = np.zeros((NCHP, NGRP), np.float32)
    for g in range(NGRP):
        for k in range(AGRP):
            sel[6 * g + k, g] = 1.0
    sel[79, 0:NUM_CLS] = 1.0
    sel = sel.astype(ml_dtypes.bfloat16)

    # ---- partition the 4608 samples (neg first, then pos) by owning core ----
    pos_ids = np.asarray(pos_ids)
    neg_ids = np.asarray(neg_ids)
    cls_labels = np.asarray(cls_labels)
    reg_labels = np.asarray(reg_labels, np.float32)

    all_b = np.concatenate([neg_ids[:, 0], pos_ids[:, 0]]).astype(np.int64)
    all_a = np.concatenate([neg_ids[:, 1], pos_ids[:, 1]]).astype(np.int64)
    k_of = all_a // HW                      # anchor offset within pixel, 0..5
    pos_of = all_a % HW                     # spatial position
    core_of = 2 * all_b + (pos_of >= HALF)
    col_of = pos_of % HALF                  # column within the core's shard
    is_pos = np.arange(N_SAMP) >= M_NEG
    label = cls_labels.astype(np.int64)

    # samples are gathered per gather-tile (right after that tile's pred
    # columns land in SBUF), so bucket by (core, gtile)
    gt_off = np.cumsum([0] + GTILES[:-1])
    gt_id_of = np.searchsorted(np.cumsum(GTILES), col_of, side="right")

    NGT = len(GTILES)
    bucket_counts = np.zeros((N_CORES, NGT), np.int64)
    for c in range(N_CORES):
        for si in range(NGT):
            bucket_counts[c, si] = int(
                ((core_of == c) & (gt_id_of == si)).sum())
    # shared padded bucket sizes (same graph on every core); multiples of 32 so
    # every bucket's wrapped-index slice stays 4-byte aligned for the ucode's
    # 32-bit index reads
    bsize = [max(32, _ceil_to(int(bucket_counts[:, si].max()), 32))
             for si in range(NGT)]
    boff = np.cumsum([0] + bsize[:-1])
    NS = int(sum(bsize))
    meta = {"bsize": bsize}

    in_maps = []
    slab_offs = np.cumsum([0] + SLABS[:-1])
    for c in range(N_CORES):
        b_idx, half = c // 2, c % 2
        xs = np.ascontiguousarray(
            x[b_idx, :, half * HALF:(half + 1) * HALF]).astype(FP8_NP)
        # tilt per slab so each slab's [128, 4, w] SBUF tile is one contiguous
        # DRAM region (one dma_start with 16 KiB/partition descriptors)
        xt = np.empty((128, 4 * HALF), FP8_NP)
        for soff, w in zip(slab_offs, SLABS):
            for k in range(4):
                xt[:, 4 * soff + k * w:4 * soff + (k + 1) * w] = \
                    xs[128 * k:128 * (k + 1), soff:soff + w]

        # slot each sample into its gtile bucket (order within bucket is
        # arbitrary)
        jc = np.nonzero(core_of == c)[0]
        order = np.argsort(gt_id_of[jc], kind="stable")
        j = jc[order]
        sl = gt_id_of[j]
        s = np.zeros(len(j), np.int64)      # sample slot within [0, NS)
        for si in range(NGT):
            m = sl == si
            s[m] = boff[si] + np.arange(int(m.sum()))
        n = len(j)
        cols = col_of[j]
        ks = k_of[j]

        # gather indices are gtile-relative; wrapped per 16 partitions
        gidx16 = np.zeros((16, NS // 16), np.int16)
        gidx16[s % 16, s // 16] = (cols - gt_off[sl]).astype(np.int16)
        gidx = np.tile(gidx16, (NCHP // 16, 1))

        kmask = np.zeros((NCHP, NS), np.float32)
        for g in range(NUM_CLS):
            kmask[6 * g + ks, s] = 1.0
        jp = is_pos[j]
        sp = s[jp]
        for g in range(CODE):
            kmask[24 + 6 * g + ks[jp], sp] = 1.0
        pad = np.ones(NS, bool)
        pad[s] = False
        kmask[79, pad] = float(np.log(1.0 / NUM_CLS))  # pad neutralizer

        clab = np.zeros((NUM_CLS, NS), np.float32)
        clab[label[j], s] = 1.0

        reglab = np.zeros((CODE, NS), np.float32)
        reglab[:, sp] = reg_labels[j[jp] - M_NEG].T

        in_maps.append({
            "x": xt,
            "wT": wT,
            "bias": bias_pad,
            "sel": sel,
            "gidx": gidx,
            "kmask": kmask,
            "clab": clab,
            "reglab": reglab,
        })
    return in_maps, NS, meta


def _patch_act_tables():
    """Force the act-table pass to pick the one set covering every function we
    use (ln/exp/relu/abs/square/copy/identity) so exactly one table load is
    emitted, early. Ids are positional, so blank the other sets instead of
    filtering them out."""
    if getattr(bacc, "_act_tables_patched", False):
        return
    orig = bacc.get_activation_tables

    def patched(module_arch):
        tabs = dict(orig(module_arch))
        keep = "natural_log_exp_and_others"
        if keep in tabs:
            tabs = {k: (v if k == keep else set()) for k, v in tabs.items()}
        return tabs

    bacc.get_activation_tables = patched
    bacc._act_tables_patched = True


def _build_graph(NS, meta):
    from concourse import library_config

    _patch_act_tables()
    nc = bacc.Bacc("TRN2", target_bir_lowering=False, debug=False,
                   num_devices=N_CORES)
    af = mybir.ActivationFunctionType

    xp = nc.declare_dram_parameter("x", [128, 4 * HALF], FP8, isOutput=False)
    wTp = nc.declare_dram_parameter("wT", [128, 4, NCHM], FP8, isOutput=False)
    biasp = nc.declare_dram_parameter("bias", [NCHP, 1], F32, isOutput=False)
    selp = nc.declare_dram_parameter("sel", [NCHP, NGRP], BF16, isOutput=False)
    gidxp = nc.declare_dram_parameter("gidx", [NCHP, NS // 16], mybir.dt.int16,
                                      isOutput=False)
    kmaskp = nc.declare_dram_parameter("kmask", [NCHP, NS], F32, isOutput=False)
    clabp = nc.declare_dram_parameter("clab", [NUM_CLS, NS], F32, isOutput=False)
    reglabp = nc.declare_dram_parameter("reglab", [CODE, NS], F32, isOutput=False)
    outp = nc.declare_dram_parameter("out", [1, 2], F32, isOutput=True)

    with tile.TileContext(nc) as tc:
        with (
            tc.tile_pool(name="const", bufs=1) as cpool,
            tc.tile_pool(name="xin", bufs=4) as xpool,
            tc.tile_pool(name="pred", bufs=1) as ppool,
            tc.tile_pool(name="post", bufs=1) as spool,
            tc.tile_pool(name="mmps", bufs=4, space=bass.MemorySpace.PSUM) as mps,
            tc.tile_pool(name="smps", bufs=1, space=bass.MemorySpace.PSUM) as sps,
        ):
            # load the ap_gather gpsimd ucode library up front, and fire a tiny
            # dummy gather: the Q7 cores fetch the ucode overlay lazily at the
            # first extended instruction (~17 us), so force that fetch early,
            # under the GEMM, instead of right before the real gather
            nc.gpsimd.load_library(library_config.ap_gather)
            dg_in = cpool.tile([16, 16], F32)
            nc.vector.memset(dg_in[:], 0.0)
            dg_idx = cpool.tile([16, 1], mybir.dt.int16)
            nc.vector.memset(dg_idx[:], 0)
            dg_out = cpool.tile([16, 16], F32)
            nc.gpsimd.ap_gather(dg_out[:], dg_in[:], dg_idx[:], channels=16,
                                num_elems=16, d=1, num_idxs=16)

            # activation-table prefetch: make the first ACT instruction an Exp
            # so the single covering table (ln/exp/relu/abs/square/copy) loads
            # once, early, overlapped with the first DMA slab.
            scratch = cpool.tile([1, 16], F32)
            nc.vector.memset(scratch[:], 0.0)
            nc.scalar.activation(scratch[:], scratch[:], af.Exp)

            # constants / per-core aux inputs on the ACT HWDGE ring so the x
            # slab DMAs on the sync ring are not queued behind them
            wt = cpool.tile([128, 4, NCHM], FP8)
            nc.scalar.dma_start(out=wt[:], in_=wTp[:])
            bias_t = cpool.tile([NCHP, 1], F32)
            nc.scalar.dma_start(out=bias_t[:], in_=biasp[:])
            sel_t = cpool.tile([NCHP, NGRP], BF16)
            nc.scalar.dma_start(out=sel_t[:], in_=selp[:])
            gidx_t = cpool.tile([NCHP, NS // 16], mybir.dt.int16)
            nc.scalar.dma_start(out=gidx_t[:], in_=gidxp[:])
            kmask_t = cpool.tile([NCHP, NS], F32)
            nc.scalar.dma_start(out=kmask_t[:], in_=kmaskp[:])
            clab_t = cpool.tile([NUM_CLS, NS], F32)
            nc.scalar.dma_start(out=clab_t[:], in_=clabp[:])
            reglab_t = cpool.tile([CODE, NS], F32)
            nc.scalar.dma_start(out=reglab_t[:], in_=reglabp[:])
            ones = cpool.tile([128, 1], BF16)
            nc.vector.memset(ones[:], 1.0)

            bsize = meta["bsize"]
            boffs = [0]
            for bs in bsize:
                boffs.append(boffs[-1] + bs)

            # single sample tiles; gathers fill G per bucket during the GEMM,
            # but nothing READS them until the end -- any mid-GEMM consumer
            # would head-of-line-block the evictions in the in-order DVE/ACT
            # queues whenever a gather runs late (Q7 cold-wake)
            G = spool.tile([NCHP, NS], F32)
            GM = spool.tile([NCHP, NS], BF16)

            nchunk = (NS + NTILE - 1) // NTILE
            # per-chunk accumulators (summed/combined at the very end)
            lse_parts = spool.tile([1, nchunk], F32)    # sum ln(sum exp(Y))
            pk_parts = spool.tile([NUM_CLS, nchunk], F32)  # sum clab*Y
            m_parts = spool.tile([CODE, nchunk], F32)   # sum (|d|-min(|d|,1))
            sq_parts = spool.tile([CODE, nchunk], F32)  # sum 0.5*min(|d|,1)^2
            onesf = cpool.tile([NCHM, 1], F32)
            nc.vector.memset(onesf[:], 1.0)
            z7 = cpool.tile([CODE, NTILE], F32)
            nc.vector.memset(z7[:], 0.0)

            # ---- dense 1x1-conv GEMM: pred = (W @ x)[:80]; evictions land in
            #      per-region pred tiles; a region's ap_gather fires once its
            #      last eviction lands ----
            gt_start = np.cumsum([0] + GTILES[:-1])
            gt_end = np.cumsum(GTILES)
            region_tiles = {}
            off = 0
            ti = 0
            for si, wdt in enumerate(SLABS):
                xt = xpool.tile([128, 4, wdt], FP8, tag="xt")
                # one contiguous DMA per slab (host pre-tilted the layout)
                nc.sync.dma_start(
                    out=xt[:],
                    in_=xp[:, 4 * off:4 * (off + wdt)].rearrange(
                        "p (k w) -> p k w", k=4))
                for t0 in range(0, wdt, NTILE):
                    tw = min(NTILE, wdt - t0)
                    gcol = off + t0
                    ri = int(np.searchsorted(gt_end, gcol, side="right"))
                    if ri not in region_tiles:
                        region_tiles[ri] = ppool.tile(
                            [NCHP, GTILES[ri]], F32, tag=f"pred{ri}",
                            name=f"pred{ri}")
                    pred_t = region_tiles[ri]
                    ps = mps.tile([NCHM, tw], F32, tag="mm")
                    for dd in range(2):
                        nc.tensor.matmul(
                            ps[:], wt[:, 2 * dd:2 * dd + 2, :],
                            xt[:, 2 * dd:2 * dd + 2, t0:t0 + tw],
                            start=(dd == 0), stop=(dd == 1),
                            perf_mode=mybir.MatmulPerfMode.DoubleRow)
                    rcol = gcol - int(gt_start[ri])
                    dst = pred_t[:, rcol:rcol + tw]
                    if ti % 2 == 0:
                        nc.vector.tensor_copy(dst, ps[0:NCHP, :])
                    else:
                        nc.scalar.copy(dst, ps[0:NCHP, :])
                    ti += 1
                    # one dummy gather mid-tail resets the Q7 idle clock so
                    # the real tail gather starts without a cold-wake penalty
                    if ri == len(GTILES) - 1 and rcol == 4096:
                        dgo = cpool.tile([16, 16], F32)
                        nc.gpsimd.ap_gather(
                            dgo[:], pred_t[0:16, rcol:rcol + 16], dg_idx[:],
                            channels=16, num_elems=16, d=1, num_idxs=16)
                    if gcol + tw == int(gt_end[ri]):
                        bs = bsize[ri]
                        bo = boffs[ri]
                        nc.gpsimd.ap_gather(
                            G[:, bo:bo + bs], pred_t[:],
                            gidx_t[:, bo // 16:(bo + bs) // 16],
                            channels=NCHP, num_elems=GTILES[ri], d=1,
                            num_idxs=bs)
                off += wdt

            # keep the PE's HAM clock warm across the trailing-gather wait so
            # the post-phase matmuls run at full clock (dead writes, no reader)
            for wi in range(16):
                wps = mps.tile([NCHM, NTILE], F32, tag="mm", name=f"warm{wi}")
                nc.tensor.matmul(wps[:], wt[:, 0:2, :], xt[:, 0:2, 0:NTILE],
                                 start=True, stop=True,
                                 perf_mode=mybir.MatmulPerfMode.DoubleRow)

            # ---- post phase: bias+mask fuse, anchor-offset selection,
            #      softmax-CE and smooth-L1 partial sums ----
            # GM = (G + bias) * kmask, cast to bf16
            nc.vector.scalar_tensor_tensor(GM[:], G[:], bias_t[:, 0:1],
                                           kmask_t[:],
                                           op0=mybir.AluOpType.add,
                                           op1=mybir.AluOpType.mult)
            for ci, c0 in enumerate(range(0, NS, NTILE)):
                cw = min(NTILE, NS - c0)
                ch = slice(c0, c0 + cw)
                ycp = sps.tile([NUM_CLS, cw], F32, tag="yc")
                nc.tensor.matmul(ycp[:], sel_t[:, 0:NUM_CLS], GM[:, ch],
                                 start=True, stop=True)
                yrp = sps.tile([CODE, cw], F32, tag="yr")
                nc.tensor.matmul(yrp[:], sel_t[:, NUM_CLS:NGRP], GM[:, ch],
                                 start=True, stop=True)
                E = spool.tile([NUM_CLS, cw], BF16, tag=f"e{ci}", name=f"E{ci}")
                PKS = spool.tile([NUM_CLS, cw], F32, tag=f"p4{ci}",
                                 name=f"PKS{ci}")
                D = spool.tile([CODE, cw], F32, tag=f"d{ci}", name=f"D{ci}")
                nc.scalar.activation(E[:], ycp[:], af.Exp)
                # sum of picked logits: accumulate (Y * clab) per partition
                nc.vector.scalar_tensor_tensor(
                    PKS[:], ycp[:], 1.0, clab_t[:, ch],
                    op0=mybir.AluOpType.mult, op1=mybir.AluOpType.mult,
                    accum_out=pk_parts[:, ci:ci + 1])
                nc.vector.tensor_sub(D[:], yrp[:], reglab_t[:, ch])
                sep = sps.tile([1, cw], F32, tag="sp", name=f"sep{ci}")
                nc.tensor.matmul(sep[:], ones[0:NUM_CLS, :], E[:],
                                 start=True, stop=True)
                lse = spool.tile([1, cw], F32, tag=f"l{ci}", name=f"lse{ci}")
                nc.scalar.activation(lse[:], sep[:], af.Ln)
                lss = spool.tile([1, cw], F32, tag=f"ls{ci}", name=f"lss{ci}")
                nc.vector.scalar_tensor_tensor(
                    lss[:], lse[:], 0.0, z7[0:1, :cw],
                    op0=mybir.AluOpType.add, op1=mybir.AluOpType.add,
                    accum_out=lse_parts[:, ci:ci + 1])
                # smooth-L1 via m=min(|d|,1): sum 0.5*m^2 + sum(|d| - m)
                AD = spool.tile([CODE, cw], F32, tag=f"ad{ci}", name=f"AD{ci}")
                M1 = spool.tile([CODE, cw], F32, tag=f"m1{ci}", name=f"M1{ci}")
                T1 = spool.tile([CODE, cw], F32, tag=f"t1{ci}", name=f"T1{ci}")
                SQ = spool.tile([CODE, cw], F32, tag=f"sq{ci}", name=f"SQ{ci}")
                nc.scalar.activation(AD[:], D[:], af.Abs)
                nc.vector.tensor_scalar_min(M1[:], AD[:], 1.0)
                nc.vector.scalar_tensor_tensor(
                    T1[:], AD[:], 0.0, M1[:],
                    op0=mybir.AluOpType.add, op1=mybir.AluOpType.subtract,
                    accum_out=m_parts[:, ci:ci + 1])
                nc.vector.scalar_tensor_tensor(
                    SQ[:], M1[:], 0.5, M1[:],
                    op0=mybir.AluOpType.mult, op1=mybir.AluOpType.mult,
                    accum_out=sq_parts[:, ci:ci + 1])

            # ---- finals: cls = sum(lse) - sum(pk);
            #      reg = sum(0.5 m^2) + sum(|d| - m)  (m_parts holds |d|-m) ----
            regc = spool.tile([CODE, nchunk], F32)
            regrow = spool.tile([CODE, 1], F32)
            nc.vector.scalar_tensor_tensor(regc[:], sq_parts[:], 0.0,
                                           m_parts[:],
                                           op0=mybir.AluOpType.add,
                                           op1=mybir.AluOpType.add,
                                           accum_out=regrow[:])
            pkrow = spool.tile([NUM_CLS, 1], F32)
            nc.vector.reduce_sum(pkrow[:], pk_parts[:],
                                 axis=mybir.AxisListType.X)
            lse_sum = spool.tile([1, 1], F32)
            nc.vector.reduce_sum(lse_sum[:], lse_parts[:],
                                 axis=mybir.AxisListType.X)
            # cross-partition sums via tiny matmuls: [reg, pk] in one pass
            rsp = sps.tile([1, 1], F32, tag="sp", name="rsp")
            nc.tensor.matmul(rsp[:], onesf[0:CODE, :], regrow[:],
                             start=True, stop=True)
            pksp = sps.tile([1, 1], F32, tag="sp2", name="pksp")
            nc.tensor.matmul(pksp[:], onesf[0:NUM_CLS, :], pkrow[:],
                             start=True, stop=True)
            cls_sum = spool.tile([1, 1], F32)
            nc.vector.tensor_sub(cls_sum[:], lse_sum[:], pksp[:])

            outb = spool.tile([1, 2], F32)
            nc.scalar.copy(outb[0:1, 0:1], cls_sum[:])
            nc.scalar.copy(outb[0:1, 1:2], rsp[:])
            nc.sync.dma_start(out=outp[:], in_=outb[:])

    nc.compile()
    return nc


def kernel(**inputs):
    global LAST_RESULT
    if os.environ.get("BASS_LDW_OPT", "0") == "1":
        _patch_ldw_opt()
    if os.environ.get("BASS_FAST_TAIL", "1") == "1":
        _patch_fast_tail()
    in_maps, NS, meta = _host_prep(**inputs)
    nc = _build_graph(NS, meta)
    trace = os.environ.get("BASS_KERNEL_TRACE", "1") == "1"
    if trace:
        _ensure_ntff_hook()
    res = run_bass_kernel_spmd(nc, in_maps, list(range(N_CORES)), trace=trace)
    LAST_RESULT = res
    cls_sum = sum(float(r["out"][0, 0]) for r in res.results)
    reg_sum = sum(float(r["out"][0, 1]) for r in res.results)
    loss = CLS_W * cls_sum / N_SAMP + REG_W * reg_sum / (M_POS * CODE)
    return np.float32(loss)
